# revision 23
# baseline (speedup 1.0000x reference)
"""MoE downsample kernel for 8 TRN2 NeuronCores — top-2 sparse version.

Host computes the gate (cheap 16x64x4 matvec) first, so the device only
computes each sample's two selected experts (~half the dense MACs). The
SPMD constraint (one program on all 8 cores) is satisfied by an
expert-major uniform schedule parameterized only by the per-expert
selection counts n_e: for expert e every row-half instance (8 cores x 2
PE row-halves = 16 instances) runs an identical pattern of chunk-runs
(run lengths = binary decomposition of 2*n_e into {8,4,2}); the host
gathers the right (sample, chunk-range) input slab into each run's slot
and scatters the outputs back.

Per run of L chunks (chunk = 4 output rows = 512 px): the two PE column
halves process L/2 chunks each, tap-outer loop so one LDWEIGHTS per
(tap, quadrant) covers L/2 matmuls (trailing matmuls set ldweights=False
to reuse the loaded stationary operand). BN + conv-bias + GELU fused
into the ScalarE PSUM eviction; outputs written bf16; gate weighting and
top-2 concat on host.
"""

import numpy as np
import ml_dtypes

KS = [3, 5, 7, 9]
DS = [1, 2, 3, 4]
BN_EPS = 1e-5
B, CIN, H, W = 16, 64, 256, 256
CE = 64
PAD = 16
HP = WP = PAD + 256 + 15   # 287
HO = WO = 128
NCORES = 8
NTAPS = sum(k * k for k in KS)  # 164
NCHUNKS = 32                    # 4 output rows per chunk
EXPERT_ORDER = [3, 2, 0, 1]     # heavy first; light-DMA e1 last
USE_LDW_SKIP = True

_SLOT_BASE = np.cumsum([0] + [k * k for k in KS]).tolist()

# Per-expert slab geometry. Experts with even dilation (e1 d=2, e3 d=4)
# only ever read even rows/cols of the padded image (offsets and strides
# all even), so the host pre-decimates those slabs 2x in each dim.
# ro/co ranges: ro = d*u - pad + PAD over u in [0,k).
_GEOM = {}
for _e in range(4):
    _k, _d = KS[_e], DS[_e]
    _pad = _d * (_k - 1) // 2
    _ro = [_d * _u - _pad + PAD for _u in range(_k)]
    _rs = 2 if _d % 2 == 0 else 1      # host decimation factor
    _ro_min, _ro_max = min(_ro), max(_ro)
    # slab rows for a run of L chunks (in decimated units)
    # original rows [ro_min, 8(L-1)+ro_max+6]
    _GEOM[_e] = {
        "rs": _rs,
        "ro_min": _ro_min,
        "rows": lambda L, a=_ro_min, b=_ro_max, r=_rs: (8 * (L - 1) + b + 6 - a) // r + 1,
        "cols": (_ro_max - _ro_min + 2 * (WO - 1)) // _rs + 1,
    }
_MAXROWS = max(_GEOM[e]["rows"](8) for e in range(4))
_MAXCOLS = max(_GEOM[e]["cols"] for e in range(4))

_COMPILED = {}


def _tap_offsets(e):
    """(slot, row_off, col_off) in padded slab coords for expert e."""
    k, d = KS[e], DS[e]
    pad = d * (k - 1) // 2
    for u in range(k):
        for v in range(k):
            slot = _SLOT_BASE[e] + u * k + v
            yield slot, d * u - pad + PAD, d * v - pad + PAD


def _pattern(two_n):
    """Run lengths (each in {2,4,8}, smallest first) summing to 2*n_e."""
    out = []
    if two_n & 2:
        out.append(2)
    if two_n & 4:
        out.append(4)
    out += [8] * ((two_n - sum(out)) // 8)
    assert sum(out) == two_n
    return out


def _gate(x, gate_w, gate_b):
    pooled = x.astype(np.float64).mean(axis=(2, 3)).astype(np.float32)
    logits = pooled @ gate_w.T.astype(np.float32) + gate_b
    z = logits - logits.max(axis=1, keepdims=True)
    ez = np.exp(z.astype(np.float32))
    gates = ez / ez.sum(axis=1, keepdims=True)
    idx = np.argsort(-gates, axis=1, kind="stable")[:, :2]
    wsel = np.take_along_axis(gates, idx, axis=1)
    wsel = wsel / (wsel.sum(axis=1, keepdims=True) + 1e-8)
    return idx, wsel.astype(np.float32)


def _plan(idx):
    """Build the uniform schedule + per-instance run assignment.

    Returns (key, patterns, runseq, units, assign):
      key      — cache key for the compiled program (depends on n only)
      patterns — {e: [run lengths]}
      runseq   — [(e, L, rowoff, rows)] in program order
      units    — [(e, pos, rh, j, L2)] out-unit order (per core)
      assign   — {(instance 0..15, e, pos): (sample, c0)}
    """
    n = [0, 0, 0, 0]
    samples_e = {e: [] for e in range(4)}
    for s in range(B):
        for e in idx[s]:
            n[e] += 1
            samples_e[e].append(s)
    patterns = {e: _pattern(2 * n[e]) for e in range(4)}
    key = tuple(n)

    # global run inventory per (e, L): cut samples into within-sample runs
    assign = {}
    for e in EXPERT_ORDER:
        pat = patterns[e]
        need = {}
        for L in pat:
            need[L] = need.get(L, 0) + 16
        cursor = 0
        inventory = {L: [] for L in need}
        for L in sorted(need, reverse=True):
            ns = need[L] * L // NCHUNKS     # samples consumed at this L
            assert ns * NCHUNKS == need[L] * L
            for _ in range(ns):
                s = samples_e[e][cursor]
                cursor += 1
                for c0 in range(0, NCHUNKS, L):
                    inventory[L].append((s, c0))
        assert cursor == n[e], (e, cursor, n[e])
        ptr = {L: 0 for L in need}
        for pos, L in enumerate(pat):
            for i in range(16):
                assign[(i, e, pos)] = inventory[L][ptr[L]]
                ptr[L] += 1
        for L in need:
            assert ptr[L] == len(inventory[L])

    runseq = []
    units = []
    rowoff = 0
    for e in EXPERT_ORDER:
        for pos, L in enumerate(patterns[e]):
            rows = _GEOM[e]["rows"](L)
            runseq.append((e, L, rowoff, rows))
            rowoff += rows
            L2 = L // 2
            for j in range(L2):
                for rh in range(2):
                    units.append((e, pos, rh, j, L2))
    return key, patterns, runseq, units, assign


def _dedup_ldweights(blocks):
    """Drop InstLdweights that reload the stationary operand already in a
    quadrant (same weights AP + tile_position, no intervening load). The
    following matmuls (ldweights=False) then reuse the loaded weights.
    Deps of a dropped load are merged into its paired matmul."""
    ndrop = 0
    for bb, insts in blocks.items():
        last = {}
        keep = []
        i = 0
        while i < len(insts):
            inst = insts[i]
            if type(inst).__name__ == "InstLdweights":
                ap = inst.ins[0]
                tp = str(getattr(inst, "tile_position", None))
                sig = (str(ap), str(getattr(inst, "perf_mode", None)))
                nxt = insts[i + 1] if i + 1 < len(insts) else None
                if (last.get(tp) == sig and nxt is not None
                        and type(nxt).__name__ == "InstMatmult"):
                    nxt.merge_dependencies_from(inst)
                    ndrop += 1
                    i += 1
                    continue
                last[tp] = sig
            keep.append(inst)
            i += 1
        insts[:] = keep
    return ndrop


def _build_program(runseq):
    import concourse.bass as bass  # noqa: F401
    import concourse.mybir as mybir
    import concourse.tile as tile
    from concourse import bacc
    from contextlib import ExitStack

    dt = mybir.dt
    totrows = sum(r[3] for r in runseq)
    nunits = sum(2 * (r[1] // 2) for r in runseq)

    nc = bacc.Bacc("TRN2", target_bir_lowering=False, debug=False,
                   num_devices=NCORES)
    xin0 = nc.dram_tensor("xin0", [CIN, totrows, _MAXCOLS], dt.bfloat16,
                          kind="ExternalInput")
    xin1 = nc.dram_tensor("xin1", [CIN, totrows, _MAXCOLS], dt.bfloat16,
                          kind="ExternalInput")
    wt = nc.dram_tensor("wt", [CIN, NTAPS, CE], dt.bfloat16,
                        kind="ExternalInput")
    bnp = nc.dram_tensor("bnp", [CE, 4, 2], dt.float32, kind="ExternalInput")
    out = nc.dram_tensor("out", [128, nunits, 512], dt.bfloat16,
                         kind="ExternalOutput")

    orig_legalize = tile.tile_legalize

    def legalize_and_dedup(blocks, nc_):
        res = orig_legalize(blocks, nc_)
        _dedup_ldweights(res)
        return res

    tile.tile_legalize = legalize_and_dedup
    try:
        _trace_program(nc, tile, mybir, dt, runseq, xin0, xin1, wt, bnp, out)
    finally:
        tile.tile_legalize = orig_legalize

    nc.compile()
    return nc


def _trace_program(nc, tile, mybir, dt, runseq, xin0, xin1, wt, bnp, out):
    from contextlib import ExitStack

    nunits = sum(2 * (r[1] // 2) for r in runseq)
    with tile.TileContext(nc) as tc:
        with ExitStack() as ctx:
            consts = ctx.enter_context(tc.tile_pool(name="consts", bufs=1))
            slab_pool = ctx.enter_context(tc.tile_pool(name="slab", bufs=3))
            vc_pool = ctx.enter_context(tc.tile_pool(name="vc", bufs=1))
            stage_pool = ctx.enter_context(tc.tile_pool(name="st", bufs=8))
            psum_pool = ctx.enter_context(
                tc.tile_pool(name="ps", bufs=1, space="PSUM"))

            wtile = consts.tile([128, NTAPS, CE], dt.bfloat16)
            bntile = consts.tile([128, 4, 2], dt.float32)
            for half in range(2):
                p0 = half * 64
                nc.gpsimd.dma_start(out=wtile[p0:p0 + 64, :, :], in_=wt.ap())
                nc.gpsimd.dma_start(out=bntile[p0:p0 + 64, :, :], in_=bnp.ap())

            # HAM warmup: ~4us of dummy matmuls while the first slab DMAs.
            warm_ps = psum_pool.tile([128, 512], dt.float32, name="psb_0_0")
            for _ in range(10):
                nc.tensor.matmul(warm_ps[0:64, :], wtile[0:64, 0, :],
                                 wtile[0:64, 0:8, :], start=True, stop=True,
                                 tile_position=(0, 0))

            unit = 0
            for e, L, rowoff, rows in runseq:
                taps = list(_tap_offsets(e))
                T = len(taps)
                L2 = L // 2
                g = _GEOM[e]
                rs, base = g["rs"], g["ro_min"]
                cols = g["cols"]
                st = slab_pool.tile([128, _MAXROWS, _MAXCOLS], dt.bfloat16,
                                    name="st")
                nc.gpsimd.dma_start(out=st[0:64, 0:rows, 0:cols],
                                    in_=xin0[:, rowoff:rowoff + rows, 0:cols])
                nc.gpsimd.dma_start(out=st[64:128, 0:rows, 0:cols],
                                    in_=xin1[:, rowoff:rowoff + rows, 0:cols])
                ps = [[psum_pool.tile([128, 512], dt.float32,
                                      name=f"psb_{rh}_{j}")
                       for j in range(L2)] for rh in range(2)]
                def evict(rh, j, u):
                    # rh0: ScalarE gelu straight out of PSUM.
                    # rh1: VectorE copies the bank out (frees it fast, in
                    # parallel with ScalarE), ScalarE gelu later from SBUF.
                    stg = stage_pool.tile([128, 512], dt.bfloat16,
                                          name="stg")
                    if rh == 0:
                        nc.scalar.activation(
                            stg, ps[0][j],
                            mybir.ActivationFunctionType.Gelu,
                            scale=bntile[:, e, 0:1],
                            bias=bntile[:, e, 1:2])
                    else:
                        vc = vc_pool.tile([128, 512], dt.float32,
                                          name=f"vc_{j}")
                        nc.vector.tensor_copy(vc, ps[1][j])
                        nc.scalar.activation(
                            stg, vc,
                            mybir.ActivationFunctionType.Gelu,
                            scale=bntile[:, e, 0:1],
                            bias=bntile[:, e, 1:2])
                    nc.sync.dma_start(out=out[:, u, :], in_=stg)

                for t, (slot, ro, co) in enumerate(taps):
                    first = t == 0
                    last = t == T - 1
                    # chunk-index outer, quadrant inner: the PE queue is
                    # in-order, so consecutive MMs must hit different
                    # quadrants to keep all four streaming concurrently.
                    for j in range(L2):
                        for rh in range(2):
                            p0 = rh * 64
                            lhsT = wtile[p0:p0 + 64, slot, :]
                            for col in range(2):
                                q0 = col * 64
                                cc = col * L2 + j
                                rl = (8 * cc + ro - base) // rs
                                cl = (co - base) // rs
                                rstep = 2 // rs
                                rhs = st[p0:p0 + 64,
                                         rl:rl + 3 * rstep + 1:rstep,
                                         cl:cl + (WO - 1) * rstep + 1:rstep]
                                mm = nc.tensor.matmul(
                                    ps[rh][j][q0:q0 + 64, :], lhsT, rhs,
                                    start=first, stop=last,
                                    tile_position=(p0, q0))
                                if USE_LDW_SKIP and j > 0:
                                    mm.ldweights = False
                        if last:
                            # bank j (both row-halves) is complete: evict
                            # now so the bank frees while later banks'
                            # last-tap matmuls still stream.
                            evict(0, j, unit + 2 * j)
                            evict(1, j, unit + 2 * j + 1)
                unit += 2 * L2
            assert unit == nunits


def _get_program(key, runseq):
    if key not in _COMPILED:
        _COMPILED[key] = _build_program(runseq)
    return _COMPILED[key]


def _prep_weights(ws, bs, bn_scale, bn_bias, bn_mean, bn_var):
    bf16 = ml_dtypes.bfloat16
    wt = np.empty((CIN, NTAPS, CE), dtype=bf16)
    for e in range(4):
        k = KS[e]
        w = ws[e].astype(np.float32)  # [CE, CIN, k, k]
        wt[:, _SLOT_BASE[e]:_SLOT_BASE[e] + k * k, :] = (
            w.transpose(1, 2, 3, 0).reshape(CIN, k * k, CE).astype(bf16))
    inv = (bn_scale / np.sqrt(bn_var + BN_EPS)).astype(np.float32)
    shift = (np.stack(bs) * inv + bn_bias - bn_mean * inv).astype(np.float32)
    bnp = np.stack([inv, shift], axis=1)              # [4, 2, CE]
    bnp = np.ascontiguousarray(bnp.transpose(2, 0, 1))  # [CE, 4, 2]
    return wt, bnp


def run(inputs, trace=False):
    from concourse import bass_utils

    x = np.asarray(inputs["x"], dtype=np.float32)
    ws = [np.asarray(inputs[f"w{i}"], dtype=np.float32) for i in range(4)]
    bs = [np.asarray(inputs[f"b{i}"], dtype=np.float32) for i in range(4)]
    bn_scale = np.asarray(inputs["bn_scale"], dtype=np.float32)
    bn_bias = np.asarray(inputs["bn_bias"], dtype=np.float32)
    bn_mean = np.asarray(inputs["bn_mean"], dtype=np.float32)
    bn_var = np.asarray(inputs["bn_var"], dtype=np.float32)
    gate_w = np.asarray(inputs["gate_w"], dtype=np.float32)
    gate_b = np.asarray(inputs["gate_b"], dtype=np.float32)

    idx, wsel = _gate(x, gate_w, gate_b)
    key, patterns, runseq, units, assign = _plan(idx)
    nc = _get_program(key, runseq)
    wt, bnp = _prep_weights(ws, bs, bn_scale, bn_bias, bn_mean, bn_var)

    bf16 = ml_dtypes.bfloat16
    xpad = np.zeros((B, CIN, HP, WP), dtype=bf16)
    xpad[:, :, PAD:PAD + H, PAD:PAD + W] = x.astype(bf16)

    totrows = sum(r[3] for r in runseq)
    in_maps = []
    for c in range(NCORES):
        xin = np.zeros((2, CIN, totrows, _MAXCOLS), dtype=bf16)
        for rh in range(2):
            i = c * 2 + rh
            pos_ctr = {}
            for e, L, rowoff, rows in runseq:
                pos = pos_ctr.get(e, 0)
                pos_ctr[e] = pos + 1
                s, c0 = assign[(i, e, pos)]
                g = _GEOM[e]
                rs, base, cols = g["rs"], g["ro_min"], g["cols"]
                r0 = 8 * c0 + base
                xin[rh, :, rowoff:rowoff + rows, 0:cols] = (
                    xpad[s, :, r0:r0 + rows * rs:rs,
                         base:base + cols * rs:rs])
        in_maps.append({"xin0": xin[0], "xin1": xin[1],
                        "wt": wt, "bnp": bnp})

    res = bass_utils.run_bass_kernel_spmd(
        nc, in_maps, core_ids=list(range(NCORES)), trace=trace)

    # scatter device outputs -> (sample, rank) feature maps, weight, concat
    rank = {}
    for s in range(B):
        rank[(s, idx[s, 0])] = 0
        rank[(s, idx[s, 1])] = 1
    E = np.zeros((B, 2, CE, HO, WO), dtype=np.float32)
    # per-(e) position counters replicated per core (same unit order)
    for c in range(NCORES):
        o = res.results[c]["out"]  # [128, nunits, 512] bf16
        of = o.astype(np.float32)
        for u, (e, pos, rh, j, L2) in enumerate(units):
            i = c * 2 + rh
            s, c0 = assign[(i, e, pos)]
            r = rank[(s, e)]
            ca, cb = c0 + j, c0 + L2 + j
            E[s, r, :, 4 * ca:4 * ca + 4, :] = of[0:64, u].reshape(64, 4, WO)
            E[s, r, :, 4 * cb:4 * cb + 4, :] = of[64:128, u].reshape(64, 4, WO)
    outf = (E * wsel[:, :, None, None, None]).reshape(B, 2 * CE, HO, WO)
    return np.ascontiguousarray(outf), res


def kernel(**inputs):
    outf, _ = run(inputs, trace=False)
    return outf


# revision 31
# speedup vs baseline: 1.0156x; 1.0156x over previous
"""MoE downsample kernel for 8 TRN2 NeuronCores — top-2 sparse version.

Host computes the gate (cheap 16x64x4 matvec) first, so the device only
computes each sample's two selected experts (~half the dense MACs). The
SPMD constraint (one program on all 8 cores) is satisfied by an
expert-major uniform schedule parameterized only by the per-expert
selection counts n_e: for expert e every row-half instance (8 cores x 2
PE row-halves = 16 instances) runs an identical pattern of chunk-runs
(run lengths = binary decomposition of 2*n_e into {8,4,2}); the host
gathers the right (sample, chunk-range) input slab into each run's slot
and scatters the outputs back.

Per run of L chunks (chunk = 4 output rows = 512 px): the two PE column
halves process L/2 chunks each, tap-outer loop so one LDWEIGHTS per
(tap, quadrant) covers L/2 matmuls (trailing matmuls set ldweights=False
to reuse the loaded stationary operand). BN + conv-bias + GELU fused
into the ScalarE PSUM eviction; outputs written bf16; gate weighting and
top-2 concat on host.
"""

import numpy as np
import ml_dtypes

KS = [3, 5, 7, 9]
DS = [1, 2, 3, 4]
BN_EPS = 1e-5
B, CIN, H, W = 16, 64, 256, 256
CE = 64
PAD = 16
HP = WP = PAD + 256 + 15   # 287
HO = WO = 128
NCORES = 8
NTAPS = sum(k * k for k in KS)  # 164
NCHUNKS = 32                    # 4 output rows per chunk
EXPERT_ORDER = [3, 2, 0, 1]     # heavy first; light-DMA e1 last
USE_LDW_SKIP = True

_SLOT_BASE = np.cumsum([0] + [k * k for k in KS]).tolist()

# Per-expert slab geometry. Experts with even dilation (e1 d=2, e3 d=4)
# only ever read even rows/cols of the padded image (offsets and strides
# all even), so the host pre-decimates those slabs 2x in each dim.
# ro/co ranges: ro = d*u - pad + PAD over u in [0,k).
_GEOM = {}
for _e in range(4):
    _k, _d = KS[_e], DS[_e]
    _pad = _d * (_k - 1) // 2
    _ro = [_d * _u - _pad + PAD for _u in range(_k)]
    _rs = 2 if _d % 2 == 0 else 1      # host decimation factor
    _ro_min, _ro_max = min(_ro), max(_ro)
    # slab rows for a run of L chunks (in decimated units)
    # original rows [ro_min, 8(L-1)+ro_max+6]
    _GEOM[_e] = {
        "rs": _rs,
        "ro_min": _ro_min,
        "rows": lambda L, a=_ro_min, b=_ro_max, r=_rs: (8 * (L - 1) + b + 6 - a) // r + 1,
        "cols": (_ro_max - _ro_min + 2 * (WO - 1)) // _rs + 1,
    }
_MAXROWS = max(_GEOM[e]["rows"](8) for e in range(4))
_MAXCOLS = max(_GEOM[e]["cols"] for e in range(4))

_COMPILED = {}


def _tap_offsets(e):
    """(slot, row_off, col_off) in padded slab coords for expert e."""
    k, d = KS[e], DS[e]
    pad = d * (k - 1) // 2
    for u in range(k):
        for v in range(k):
            slot = _SLOT_BASE[e] + u * k + v
            yield slot, d * u - pad + PAD, d * v - pad + PAD


def _pattern(two_n):
    """Run lengths (each in {2,4,8}, smallest first) summing to 2*n_e."""
    out = []
    if two_n & 2:
        out.append(2)
    if two_n & 4:
        out.append(4)
    out += [8] * ((two_n - sum(out)) // 8)
    assert sum(out) == two_n
    return out


def _gate(x, gate_w, gate_b):
    pooled = x.astype(np.float64).mean(axis=(2, 3)).astype(np.float32)
    logits = pooled @ gate_w.T.astype(np.float32) + gate_b
    z = logits - logits.max(axis=1, keepdims=True)
    ez = np.exp(z.astype(np.float32))
    gates = ez / ez.sum(axis=1, keepdims=True)
    idx = np.argsort(-gates, axis=1, kind="stable")[:, :2]
    wsel = np.take_along_axis(gates, idx, axis=1)
    wsel = wsel / (wsel.sum(axis=1, keepdims=True) + 1e-8)
    return idx, wsel.astype(np.float32)


def _plan(idx):
    """Build the uniform schedule + per-instance run assignment.

    Returns (key, patterns, runseq, units, assign):
      key      — cache key for the compiled program (depends on n only)
      patterns — {e: [run lengths]}
      runseq   — [(e, L, rowoff, rows)] in program order
      units    — [(e, pos, rh, j, L2)] out-unit order (per core)
      assign   — {(instance 0..15, e, pos): (sample, c0)}
    """
    n = [0, 0, 0, 0]
    samples_e = {e: [] for e in range(4)}
    for s in range(B):
        for e in idx[s]:
            n[e] += 1
            samples_e[e].append(s)
    patterns = {e: _pattern(2 * n[e]) for e in range(4)}
    key = tuple(n)

    # global run inventory per (e, L): cut samples into within-sample runs
    assign = {}
    for e in EXPERT_ORDER:
        pat = patterns[e]
        need = {}
        for L in pat:
            need[L] = need.get(L, 0) + 16
        cursor = 0
        inventory = {L: [] for L in need}
        for L in sorted(need, reverse=True):
            ns = need[L] * L // NCHUNKS     # samples consumed at this L
            assert ns * NCHUNKS == need[L] * L
            for _ in range(ns):
                s = samples_e[e][cursor]
                cursor += 1
                for c0 in range(0, NCHUNKS, L):
                    inventory[L].append((s, c0))
        assert cursor == n[e], (e, cursor, n[e])
        ptr = {L: 0 for L in need}
        for pos, L in enumerate(pat):
            for i in range(16):
                assign[(i, e, pos)] = inventory[L][ptr[L]]
                ptr[L] += 1
        for L in need:
            assert ptr[L] == len(inventory[L])

    # Pair leftover small runs (L=2) of two DIFFERENT experts into one
    # "mixed" run: expert A's chunks on PE row-half 0, expert B's on
    # row-half 1 — all four quadrants stay busy and the per-tap LDWEIGHTS
    # serialization overlaps the other expert's matmuls.
    small = [(e, pos, L) for e in EXPERT_ORDER
             for pos, L in enumerate(patterns[e]) if L == 2]
    mixed = []
    used = set()
    while len(small) >= 2 and small[0][0] != small[1][0]:
        a, b = small.pop(0), small.pop(0)
        mixed.append((a, b))
        used.add((a[0], a[1]))
        used.add((b[0], b[1]))

    # runseq entries: (kind, payload, rowoff, rows)
    #   kind 'n': payload = (e, pos, L)
    #   kind 'm': payload = ((eA, posA, LA), (eB, posB, LB))
    runseq = []
    units = []
    rowoff = 0
    for e in EXPERT_ORDER:
        for pos, L in enumerate(patterns[e]):
            if (e, pos) in used:
                continue
            rows = _GEOM[e]["rows"](L)
            runseq.append(("n", (e, pos, L), rowoff, rows))
            rowoff += rows
            L2 = L // 2
            for j in range(L2):
                for rh in range(2):
                    units.append((e, pos, rh, j, L2))
    for a, b in mixed:
        # each mixed run consumes one supply-run per side per core; the
        # 16-instance supply needs two mixed runs per core (m = 0, 1).
        # units store m in the rh slot: instance = core*2 + m.
        for m in range(2):
            rows = max(_GEOM[a[0]]["rows"](a[2]), _GEOM[b[0]]["rows"](b[2]))
            runseq.append(("m", (a, b, m), rowoff, rows))
            rowoff += rows
            units.append((a[0], a[1], m, 0, 1))
            units.append((b[0], b[1], m, 0, 1))
    return key, patterns, runseq, units, assign


def _dedup_ldweights(blocks):
    """Drop InstLdweights that reload the stationary operand already in a
    quadrant (same weights AP + tile_position, no intervening load). The
    following matmuls (ldweights=False) then reuse the loaded weights.
    Deps of a dropped load are merged into its paired matmul."""
    ndrop = 0
    for bb, insts in blocks.items():
        last = {}
        keep = []
        i = 0
        while i < len(insts):
            inst = insts[i]
            if type(inst).__name__ == "InstLdweights":
                ap = inst.ins[0]
                tp = str(getattr(inst, "tile_position", None))
                sig = (str(ap), str(getattr(inst, "perf_mode", None)))
                nxt = insts[i + 1] if i + 1 < len(insts) else None
                if (last.get(tp) == sig and nxt is not None
                        and type(nxt).__name__ == "InstMatmult"):
                    nxt.merge_dependencies_from(inst)
                    ndrop += 1
                    i += 1
                    continue
                last[tp] = sig
            keep.append(inst)
            i += 1
        insts[:] = keep
    return ndrop


def _build_program(runseq):
    import concourse.bass as bass  # noqa: F401
    import concourse.mybir as mybir
    import concourse.tile as tile
    from concourse import bacc
    from contextlib import ExitStack

    dt = mybir.dt
    totrows = sum(r[3] for r in runseq)
    nunits = sum(2 * (r[1][2] // 2) if r[0] == "n" else 2 for r in runseq)

    nc = bacc.Bacc("TRN2", target_bir_lowering=False, debug=False,
                   num_devices=NCORES)
    xin0 = nc.dram_tensor("xin0", [CIN, totrows, _MAXCOLS], dt.bfloat16,
                          kind="ExternalInput")
    xin1 = nc.dram_tensor("xin1", [CIN, totrows, _MAXCOLS], dt.bfloat16,
                          kind="ExternalInput")
    wt = nc.dram_tensor("wt", [CIN, NTAPS, CE], dt.bfloat16,
                        kind="ExternalInput")
    bnp = nc.dram_tensor("bnp", [CE, 4, 2], dt.float32, kind="ExternalInput")
    out = nc.dram_tensor("out", [128, nunits, 512], dt.bfloat16,
                         kind="ExternalOutput")

    orig_legalize = tile.tile_legalize

    def legalize_and_dedup(blocks, nc_):
        res = orig_legalize(blocks, nc_)
        _dedup_ldweights(res)
        return res

    tile.tile_legalize = legalize_and_dedup
    try:
        _trace_program(nc, tile, mybir, dt, runseq, xin0, xin1, wt, bnp, out)
    finally:
        tile.tile_legalize = orig_legalize

    nc.compile()
    return nc


def _trace_program(nc, tile, mybir, dt, runseq, xin0, xin1, wt, bnp, out):
    from contextlib import ExitStack

    nunits = sum(2 * (r[1][2] // 2) if r[0] == "n" else 2 for r in runseq)
    with tile.TileContext(nc) as tc:
        with ExitStack() as ctx:
            consts = ctx.enter_context(tc.tile_pool(name="consts", bufs=1))
            slab_pool = ctx.enter_context(tc.tile_pool(name="slab", bufs=3))
            vc_pool = ctx.enter_context(tc.tile_pool(name="vc", bufs=1))
            stage_pool = ctx.enter_context(tc.tile_pool(name="st", bufs=8))
            psum_pool = ctx.enter_context(
                tc.tile_pool(name="ps", bufs=1, space="PSUM"))

            wtile = consts.tile([128, NTAPS, CE], dt.bfloat16)
            bntile = consts.tile([128, 4, 2], dt.float32)
            # scalar-queue DMA: weights load in parallel with the first
            # slab on the gpsimd queue, so compute starts sooner.
            for half in range(2):
                p0 = half * 64
                nc.scalar.dma_start(out=wtile[p0:p0 + 64, :, :], in_=wt.ap())
                nc.scalar.dma_start(out=bntile[p0:p0 + 64, :, :],
                                    in_=bnp.ap())

            # HAM warmup: ~4us of dummy matmuls while the first slab DMAs.
            warm_ps = psum_pool.tile([128, 512], dt.float32, name="psb_0_0")
            for _ in range(10):
                nc.tensor.matmul(warm_ps[0:64, :], wtile[0:64, 0, :],
                                 wtile[0:64, 0:8, :], start=True, stop=True,
                                 tile_position=(0, 0))

            def evict(ps_tile, e, free_fast, u):
                # free_fast: VectorE copies the bank out (frees it fast,
                # in parallel with ScalarE's gelu chain), ScalarE gelu
                # later from SBUF. Else ScalarE gelu straight from PSUM.
                stg = stage_pool.tile([128, 512], dt.bfloat16, name="stg")
                if not free_fast:
                    nc.scalar.activation(
                        stg, ps_tile,
                        mybir.ActivationFunctionType.Gelu,
                        scale=bntile[:, e, 0:1], bias=bntile[:, e, 1:2])
                else:
                    vc = vc_pool.tile([128, 512], dt.float32,
                                      name=f"vc_{u % 4}")
                    nc.vector.tensor_scalar_add(vc, ps_tile, 0.0)
                    nc.scalar.activation(
                        stg, vc,
                        mybir.ActivationFunctionType.Gelu,
                        scale=bntile[:, e, 0:1], bias=bntile[:, e, 1:2])
                nc.sync.dma_start(out=out[:, u, :], in_=stg)

            def mm_quad(ps_tile, st, e, slot, ro, co, cc, rh, col,
                        first, last, skip_ldw):
                g = _GEOM[e]
                rs, base = g["rs"], g["ro_min"]
                p0, q0 = rh * 64, col * 64
                rl = (8 * cc + ro - base) // rs
                cl = (co - base) // rs
                rstep = 2 // rs
                rhs = st[p0:p0 + 64,
                         rl:rl + 3 * rstep + 1:rstep,
                         cl:cl + (WO - 1) * rstep + 1:rstep]
                mm = nc.tensor.matmul(
                    ps_tile[q0:q0 + 64, :], wtile[p0:p0 + 64, slot, :],
                    rhs, start=first, stop=last, tile_position=(p0, q0))
                if USE_LDW_SKIP and skip_ldw:
                    mm.ldweights = False

            unit = 0
            for kind, payload, rowoff, rows in runseq:
                st = slab_pool.tile([128, _MAXROWS, _MAXCOLS], dt.bfloat16,
                                    name="st")
                if kind == "n":
                    e, pos, L = payload
                    taps = list(_tap_offsets(e))
                    T = len(taps)
                    L2 = L // 2
                    cols = _GEOM[e]["cols"]
                    nc.gpsimd.dma_start(
                        out=st[0:64, 0:rows, 0:cols],
                        in_=xin0[:, rowoff:rowoff + rows, 0:cols])
                    nc.gpsimd.dma_start(
                        out=st[64:128, 0:rows, 0:cols],
                        in_=xin1[:, rowoff:rowoff + rows, 0:cols])
                    ps = [[psum_pool.tile([128, 512], dt.float32,
                                          name=f"psb_{rh}_{j}")
                           for j in range(L2)] for rh in range(2)]
                    for t, (slot, ro, co) in enumerate(taps):
                        first = t == 0
                        last = t == T - 1
                        # chunk-index outer, quadrant inner: the PE queue
                        # is in-order; consecutive MMs must hit different
                        # quadrants to keep all four streaming.
                        for j in range(L2):
                            for rh in range(2):
                                for col in range(2):
                                    mm_quad(ps[rh][j], st, e, slot, ro, co,
                                            col * L2 + j, rh, col,
                                            first, last, j > 0)
                            if last:
                                # bank j complete on both row-halves:
                                # evict now so it frees while later banks'
                                # last-tap matmuls still stream.
                                evict(ps[0][j], e, False, unit + 2 * j)
                                evict(ps[1][j], e, True, unit + 2 * j + 1)
                    unit += 2 * L2
                else:
                    (eA, posA, LA), (eB, posB, LB), m = payload
                    sides = [(0, eA), (1, eB)]
                    tapsS = {0: list(_tap_offsets(eA)),
                             1: list(_tap_offsets(eB))}
                    for rh, e_ in sides:
                        cols = _GEOM[e_]["cols"]
                        rws = _GEOM[e_]["rows"](2)
                        src = xin0 if rh == 0 else xin1
                        nc.gpsimd.dma_start(
                            out=st[rh * 64:rh * 64 + 64, 0:rws, 0:cols],
                            in_=src[:, rowoff:rowoff + rws, 0:cols])
                    ps = [psum_pool.tile([128, 512], dt.float32,
                                         name=f"psb_{rh}_0")
                          for rh in range(2)]
                    Tmax = max(len(tapsS[0]), len(tapsS[1]))
                    for t in range(Tmax):
                        for rh, e_ in sides:
                            tl = tapsS[rh]
                            if t >= len(tl):
                                continue
                            slot, ro, co = tl[t]
                            first = t == 0
                            last = t == len(tl) - 1
                            for col in range(2):
                                mm_quad(ps[rh], st, e_, slot, ro, co,
                                        col, rh, col, first, last, False)
                            if last:
                                evict(ps[rh], e_, rh == 1, unit + rh)
                    unit += 2
            assert unit == nunits


def _get_program(key, runseq):
    if key not in _COMPILED:
        _COMPILED[key] = _build_program(runseq)
    return _COMPILED[key]


def _prep_weights(ws, bs, bn_scale, bn_bias, bn_mean, bn_var):
    bf16 = ml_dtypes.bfloat16
    wt = np.empty((CIN, NTAPS, CE), dtype=bf16)
    for e in range(4):
        k = KS[e]
        w = ws[e].astype(np.float32)  # [CE, CIN, k, k]
        wt[:, _SLOT_BASE[e]:_SLOT_BASE[e] + k * k, :] = (
            w.transpose(1, 2, 3, 0).reshape(CIN, k * k, CE).astype(bf16))
    inv = (bn_scale / np.sqrt(bn_var + BN_EPS)).astype(np.float32)
    shift = (np.stack(bs) * inv + bn_bias - bn_mean * inv).astype(np.float32)
    bnp = np.stack([inv, shift], axis=1)              # [4, 2, CE]
    bnp = np.ascontiguousarray(bnp.transpose(2, 0, 1))  # [CE, 4, 2]
    return wt, bnp


def run(inputs, trace=False):
    from concourse import bass_utils

    x = np.asarray(inputs["x"], dtype=np.float32)
    ws = [np.asarray(inputs[f"w{i}"], dtype=np.float32) for i in range(4)]
    bs = [np.asarray(inputs[f"b{i}"], dtype=np.float32) for i in range(4)]
    bn_scale = np.asarray(inputs["bn_scale"], dtype=np.float32)
    bn_bias = np.asarray(inputs["bn_bias"], dtype=np.float32)
    bn_mean = np.asarray(inputs["bn_mean"], dtype=np.float32)
    bn_var = np.asarray(inputs["bn_var"], dtype=np.float32)
    gate_w = np.asarray(inputs["gate_w"], dtype=np.float32)
    gate_b = np.asarray(inputs["gate_b"], dtype=np.float32)

    idx, wsel = _gate(x, gate_w, gate_b)
    key, patterns, runseq, units, assign = _plan(idx)
    nc = _get_program(key, runseq)
    wt, bnp = _prep_weights(ws, bs, bn_scale, bn_bias, bn_mean, bn_var)

    bf16 = ml_dtypes.bfloat16
    xpad = np.zeros((B, CIN, HP, WP), dtype=bf16)
    xpad[:, :, PAD:PAD + H, PAD:PAD + W] = x.astype(bf16)

    totrows = sum(r[3] for r in runseq)

    in_maps = []
    for c in range(NCORES):
        xin = np.zeros((2, CIN, totrows, _MAXCOLS), dtype=bf16)
        for kind, payload, rowoff, rows in runseq:
            if kind == "n":
                e, pos, L = payload
                g = _GEOM[e]
                rs, base, cols = g["rs"], g["ro_min"], g["cols"]
                for rh in range(2):
                    s, c0 = assign[(c * 2 + rh, e, pos)]
                    r0 = 8 * c0 + base
                    xin[rh, :, rowoff:rowoff + rows, 0:cols] = (
                        xpad[s, :, r0:r0 + rows * rs:rs,
                             base:base + cols * rs:rs])
            else:
                (eA, posA, LA), (eB, posB, LB), m = payload
                for rh, (e_, pos_, L_) in ((0, (eA, posA, LA)),
                                           (1, (eB, posB, LB))):
                    g = _GEOM[e_]
                    rs, base, cols = g["rs"], g["ro_min"], g["cols"]
                    rws = g["rows"](L_)
                    s, c0 = assign[(c * 2 + m, e_, pos_)]
                    r0 = 8 * c0 + base
                    xin[rh, :, rowoff:rowoff + rws, 0:cols] = (
                        xpad[s, :, r0:r0 + rws * rs:rs,
                             base:base + cols * rs:rs])
        in_maps.append({"xin0": xin[0], "xin1": xin[1],
                        "wt": wt, "bnp": bnp})

    res = bass_utils.run_bass_kernel_spmd(
        nc, in_maps, core_ids=list(range(NCORES)), trace=trace)

    # scatter device outputs -> (sample, rank) feature maps, weight, concat
    rank = {}
    for s in range(B):
        rank[(s, idx[s, 0])] = 0
        rank[(s, idx[s, 1])] = 1
    E = np.zeros((B, 2, CE, HO, WO), dtype=np.float32)
    # per-(e) position counters replicated per core (same unit order)
    for c in range(NCORES):
        o = res.results[c]["out"]  # [128, nunits, 512] bf16
        of = o.astype(np.float32)
        for u, (e, pos, rh, j, L2) in enumerate(units):
            i = c * 2 + rh
            s, c0 = assign[(i, e, pos)]
            r = rank[(s, e)]
            ca, cb = c0 + j, c0 + L2 + j
            E[s, r, :, 4 * ca:4 * ca + 4, :] = of[0:64, u].reshape(64, 4, WO)
            E[s, r, :, 4 * cb:4 * cb + 4, :] = of[64:128, u].reshape(64, 4, WO)
    outf = (E * wsel[:, :, None, None, None]).reshape(B, 2 * CE, HO, WO)
    return np.ascontiguousarray(outf), res


def kernel(**inputs):
    outf, _ = run(inputs, trace=False)
    return outf


# revision 38
# speedup vs baseline: 1.0372x; 1.0213x over previous
"""MoE downsample kernel for 8 TRN2 NeuronCores — top-2 sparse version.

Host computes the gate (cheap 16x64x4 matvec) first, so the device only
computes each sample's two selected experts (~half the dense MACs). The
SPMD constraint (one program on all 8 cores) is satisfied by an
expert-major uniform schedule parameterized only by the per-expert
selection counts n_e: for expert e every row-half instance (8 cores x 2
PE row-halves = 16 instances) runs an identical pattern of chunk-runs
(run lengths = binary decomposition of 2*n_e into {8,4,2}); the host
gathers the right (sample, chunk-range) input slab into each run's slot
and scatters the outputs back.

Per run of L chunks (chunk = 4 output rows = 512 px): the two PE column
halves process L/2 chunks each, tap-outer loop so one LDWEIGHTS per
(tap, quadrant) covers L/2 matmuls (trailing matmuls set ldweights=False
to reuse the loaded stationary operand). BN + conv-bias + GELU fused
into the ScalarE PSUM eviction; outputs written bf16; gate weighting and
top-2 concat on host.
"""

import numpy as np
import ml_dtypes

KS = [3, 5, 7, 9]
DS = [1, 2, 3, 4]
BN_EPS = 1e-5
B, CIN, H, W = 16, 64, 256, 256
CE = 64
PAD = 16
HP = WP = PAD + 256 + 15   # 287
HO = WO = 128
NCORES = 8
NTAPS = sum(k * k for k in KS)  # 164
NCHUNKS = 32                    # 4 output rows per chunk
EXPERT_ORDER = [3, 2, 0, 1]     # heavy first; light-DMA e1 last
USE_LDW_SKIP = True

_SLOT_BASE = np.cumsum([0] + [k * k for k in KS]).tolist()

# Per-expert slab geometry. Experts with even dilation (e1 d=2, e3 d=4)
# only ever read even rows/cols of the padded image (offsets and strides
# all even), so the host pre-decimates those slabs 2x in each dim.
# ro/co ranges: ro = d*u - pad + PAD over u in [0,k).
_GEOM = {}
for _e in range(4):
    _k, _d = KS[_e], DS[_e]
    _pad = _d * (_k - 1) // 2
    _ro = [_d * _u - _pad + PAD for _u in range(_k)]
    _rs = 2 if _d % 2 == 0 else 1      # host decimation factor
    _ro_min, _ro_max = min(_ro), max(_ro)
    # slab rows for a run of L chunks (in decimated units)
    # original rows [ro_min, 8(L-1)+ro_max+6]
    _GEOM[_e] = {
        "rs": _rs,
        "ro_min": _ro_min,
        "rows": lambda L, a=_ro_min, b=_ro_max, r=_rs: (8 * (L - 1) + b + 6 - a) // r + 1,
        "cols": (_ro_max - _ro_min + 2 * (WO - 1)) // _rs + 1,
    }
_MAXROWS = max(_GEOM[e]["rows"](8) for e in range(4))
_MAXCOLS = max(_GEOM[e]["cols"] for e in range(4))

_COMPILED = {}


def _tap_offsets(e):
    """(slot, row_off, col_off) in padded slab coords for expert e."""
    k, d = KS[e], DS[e]
    pad = d * (k - 1) // 2
    for u in range(k):
        for v in range(k):
            slot = _SLOT_BASE[e] + u * k + v
            yield slot, d * u - pad + PAD, d * v - pad + PAD


def _pattern(two_n):
    """Run lengths (each in {2,4,8}, smallest first) summing to 2*n_e."""
    out = []
    if two_n & 2:
        out.append(2)
    if two_n & 4:
        out.append(4)
    out += [8] * ((two_n - sum(out)) // 8)
    assert sum(out) == two_n
    return out


def _gate(x, gate_w, gate_b):
    pooled = x.astype(np.float64).mean(axis=(2, 3)).astype(np.float32)
    logits = pooled @ gate_w.T.astype(np.float32) + gate_b
    z = logits - logits.max(axis=1, keepdims=True)
    ez = np.exp(z.astype(np.float32))
    gates = ez / ez.sum(axis=1, keepdims=True)
    idx = np.argsort(-gates, axis=1, kind="stable")[:, :2]
    wsel = np.take_along_axis(gates, idx, axis=1)
    wsel = wsel / (wsel.sum(axis=1, keepdims=True) + 1e-8)
    return idx, wsel.astype(np.float32)


def _plan(idx):
    """Build the uniform schedule + per-instance run assignment.

    Returns (key, patterns, runseq, units, assign):
      key      — cache key for the compiled program (depends on n only)
      patterns — {e: [run lengths]}
      runseq   — [(e, L, rowoff, rows)] in program order
      units    — [(e, pos, rh, j, L2)] out-unit order (per core)
      assign   — {(instance 0..15, e, pos): (sample, c0)}
    """
    n = [0, 0, 0, 0]
    samples_e = {e: [] for e in range(4)}
    for s in range(B):
        for e in idx[s]:
            n[e] += 1
            samples_e[e].append(s)
    patterns = {e: _pattern(2 * n[e]) for e in range(4)}
    key = tuple(n)

    # global run inventory per (e, L): cut samples into within-sample runs
    assign = {}
    for e in EXPERT_ORDER:
        pat = patterns[e]
        need = {}
        for L in pat:
            need[L] = need.get(L, 0) + 16
        cursor = 0
        inventory = {L: [] for L in need}
        for L in sorted(need, reverse=True):
            ns = need[L] * L // NCHUNKS     # samples consumed at this L
            assert ns * NCHUNKS == need[L] * L
            for _ in range(ns):
                s = samples_e[e][cursor]
                cursor += 1
                for c0 in range(0, NCHUNKS, L):
                    inventory[L].append((s, c0))
        assert cursor == n[e], (e, cursor, n[e])
        ptr = {L: 0 for L in need}
        for pos, L in enumerate(pat):
            for i in range(16):
                assign[(i, e, pos)] = inventory[L][ptr[L]]
                ptr[L] += 1
        for L in need:
            assert ptr[L] == len(inventory[L])

    # Pair leftover small runs (L=2) of two DIFFERENT experts into one
    # "mixed" run: expert A's chunks on PE row-half 0, expert B's on
    # row-half 1 — all four quadrants stay busy and the per-tap LDWEIGHTS
    # serialization overlaps the other expert's matmuls.
    small = [(e, pos, L) for e in EXPERT_ORDER
             for pos, L in enumerate(patterns[e]) if L == 2]
    mixed = []
    used = set()
    while len(small) >= 2 and small[0][0] != small[1][0]:
        a, b = small.pop(0), small.pop(0)
        mixed.append((a, b))
        used.add((a[0], a[1]))
        used.add((b[0], b[1]))

    # runseq entries: (kind, payload, (off0, off1)) — offsets into the
    # flat per-row-half HBM input arrays (elements, per partition).
    #   kind 'n': payload = (e, pos, L)
    #   kind 'm': payload = ((eA, posA, LA), (eB, posB, LB), m)
    runseq = []
    units = []
    offs = [0, 0]
    for e in EXPERT_ORDER:
        for pos, L in enumerate(patterns[e]):
            if (e, pos) in used:
                continue
            elems = _GEOM[e]["rows"](L) * _GEOM[e]["cols"]
            runseq.append(("n", (e, pos, L), tuple(offs)))
            offs[0] += elems
            offs[1] += elems
            L2 = L // 2
            for j in range(L2):
                for rh in range(2):
                    units.append((e, pos, rh, j, L2))
    for a, b in mixed:
        # each mixed run consumes one supply-run per side per core; the
        # 16-instance supply needs two mixed runs per core (m = 0, 1).
        # units store m in the rh slot: instance = core*2 + m.
        for m in range(2):
            runseq.append(("m", (a, b, m), tuple(offs)))
            offs[0] += _GEOM[a[0]]["rows"](a[2]) * _GEOM[a[0]]["cols"]
            offs[1] += _GEOM[b[0]]["rows"](b[2]) * _GEOM[b[0]]["cols"]
            units.append((a[0], a[1], m, 0, 1))
            units.append((b[0], b[1], m, 0, 1))
    return key, patterns, runseq, units, assign, tuple(offs)


def _dedup_ldweights(blocks):
    """Drop InstLdweights that reload the stationary operand already in a
    quadrant (same weights AP + tile_position, no intervening load). The
    following matmuls (ldweights=False) then reuse the loaded weights.
    Deps of a dropped load are merged into its paired matmul."""
    ndrop = 0
    for bb, insts in blocks.items():
        last = {}
        keep = []
        i = 0
        while i < len(insts):
            inst = insts[i]
            if type(inst).__name__ == "InstLdweights":
                ap = inst.ins[0]
                tp = str(getattr(inst, "tile_position", None))
                sig = (str(ap), str(getattr(inst, "perf_mode", None)))
                nxt = insts[i + 1] if i + 1 < len(insts) else None
                if (last.get(tp) == sig and nxt is not None
                        and type(nxt).__name__ == "InstMatmult"):
                    nxt.merge_dependencies_from(inst)
                    ndrop += 1
                    i += 1
                    continue
                last[tp] = sig
            keep.append(inst)
            i += 1
        insts[:] = keep
    return ndrop


def _build_program(runseq, tot_elems):
    import concourse.bass as bass  # noqa: F401
    import concourse.mybir as mybir
    import concourse.tile as tile
    from concourse import bacc
    from contextlib import ExitStack

    dt = mybir.dt
    nunits = sum(2 * (r[1][2] // 2) if r[0] == "n" else 2 for r in runseq)

    nc = bacc.Bacc("TRN2", target_bir_lowering=False, debug=False,
                   num_devices=NCORES)
    xin0 = nc.dram_tensor("xin0", [CIN, tot_elems[0]], dt.bfloat16,
                          kind="ExternalInput")
    xin1 = nc.dram_tensor("xin1", [CIN, tot_elems[1]], dt.bfloat16,
                          kind="ExternalInput")
    wt = nc.dram_tensor("wt", [CIN, NTAPS, CE], dt.bfloat16,
                        kind="ExternalInput")
    bnp = nc.dram_tensor("bnp", [CE, 4, 2], dt.float32, kind="ExternalInput")
    out = nc.dram_tensor("out", [128, nunits, 512], dt.bfloat16,
                         kind="ExternalOutput")

    orig_legalize = tile.tile_legalize

    def legalize_and_dedup(blocks, nc_):
        res = orig_legalize(blocks, nc_)
        _dedup_ldweights(res)
        return res

    tile.tile_legalize = legalize_and_dedup
    try:
        _trace_program(nc, tile, mybir, dt, runseq, xin0, xin1, wt, bnp, out)
    finally:
        tile.tile_legalize = orig_legalize

    nc.compile()
    return nc


def _trace_program(nc, tile, mybir, dt, runseq, xin0, xin1, wt, bnp, out):
    from contextlib import ExitStack

    nunits = sum(2 * (r[1][2] // 2) if r[0] == "n" else 2 for r in runseq)
    with tile.TileContext(nc) as tc:
        with ExitStack() as ctx:
            consts = ctx.enter_context(tc.tile_pool(name="consts", bufs=1))
            slab_pool = ctx.enter_context(tc.tile_pool(name="slab", bufs=3))
            vc_pool = ctx.enter_context(tc.tile_pool(name="vc", bufs=1))
            stage_pool = ctx.enter_context(tc.tile_pool(name="st", bufs=8))
            psum_pool = ctx.enter_context(
                tc.tile_pool(name="ps", bufs=1, space="PSUM"))

            wtile = consts.tile([128, NTAPS, CE], dt.bfloat16)
            bntile = consts.tile([128, 4, 2], dt.float32)
            # scalar-queue DMA: weights load in parallel with the first
            # slab on the gpsimd queue, so compute starts sooner.
            for half in range(2):
                p0 = half * 64
                nc.scalar.dma_start(out=wtile[p0:p0 + 64, :, :], in_=wt.ap())
                nc.scalar.dma_start(out=bntile[p0:p0 + 64, :, :],
                                    in_=bnp.ap())

            # HAM warmup: ~4us of dummy matmuls while the first slab DMAs.
            warm_ps = psum_pool.tile([128, 512], dt.float32, name="psb_0_0")
            for _ in range(10):
                nc.tensor.matmul(warm_ps[0:64, :], wtile[0:64, 0, :],
                                 wtile[0:64, 0:8, :], start=True, stop=True,
                                 tile_position=(0, 0))

            def evict(ps_tile, e, free_fast, u):
                # free_fast: VectorE copies the bank out (frees it fast,
                # in parallel with ScalarE's gelu chain), ScalarE gelu
                # later from SBUF. Else ScalarE gelu straight from PSUM.
                stg = stage_pool.tile([128, 512], dt.bfloat16, name="stg")
                if not free_fast:
                    nc.scalar.activation(
                        stg, ps_tile,
                        mybir.ActivationFunctionType.Gelu,
                        scale=bntile[:, e, 0:1], bias=bntile[:, e, 1:2])
                else:
                    vc = vc_pool.tile([128, 512], dt.float32,
                                      name=f"vc_{u % 4}")
                    nc.vector.tensor_scalar_add(vc, ps_tile, 0.0)
                    nc.scalar.activation(
                        stg, vc,
                        mybir.ActivationFunctionType.Gelu,
                        scale=bntile[:, e, 0:1], bias=bntile[:, e, 1:2])
                nc.sync.dma_start(out=out[:, u, :], in_=stg)

            def mm_quad(ps_tile, st, e, slot, ro, co, cc, rh, col,
                        first, last, skip_ldw):
                g = _GEOM[e]
                rs, base = g["rs"], g["ro_min"]
                p0, q0 = rh * 64, col * 64
                rl = (8 * cc + ro - base) // rs
                cl = (co - base) // rs
                rstep = 2 // rs
                rhs = st[p0:p0 + 64,
                         rl:rl + 3 * rstep + 1:rstep,
                         cl:cl + (WO - 1) * rstep + 1:rstep]
                mm = nc.tensor.matmul(
                    ps_tile[q0:q0 + 64, :], wtile[p0:p0 + 64, slot, :],
                    rhs, start=first, stop=last, tile_position=(p0, q0))
                if USE_LDW_SKIP and skip_ldw:
                    mm.ldweights = False

            unit = 0
            for kind, payload, offs in runseq:
                if kind == "n":
                    e, pos, L = payload
                    taps = list(_tap_offsets(e))
                    T = len(taps)
                    L2 = L // 2
                    cols = _GEOM[e]["cols"]
                    rows = _GEOM[e]["rows"](L)
                    ne = rows * cols
                    # tight tile: contiguous per-partition DMA block
                    st = slab_pool.tile([128, rows, cols], dt.bfloat16,
                                        name="st")
                    nc.gpsimd.dma_start(
                        out=st[0:64, :, :],
                        in_=xin0[:, offs[0]:offs[0] + ne])
                    nc.gpsimd.dma_start(
                        out=st[64:128, :, :],
                        in_=xin1[:, offs[1]:offs[1] + ne])
                    ps = [[psum_pool.tile([128, 512], dt.float32,
                                          name=f"psb_{rh}_{j}")
                           for j in range(L2)] for rh in range(2)]
                    for t, (slot, ro, co) in enumerate(taps):
                        first = t == 0
                        last = t == T - 1
                        # chunk-index outer, quadrant inner: the PE queue
                        # is in-order; consecutive MMs must hit different
                        # quadrants to keep all four streaming.
                        for j in range(L2):
                            for rh in range(2):
                                for col in range(2):
                                    mm_quad(ps[rh][j], st, e, slot, ro, co,
                                            col * L2 + j, rh, col,
                                            first, last, j > 0)
                            if last:
                                # bank j complete on both row-halves:
                                # evict now so it frees while later banks'
                                # last-tap matmuls still stream.
                                evict(ps[0][j], e, False, unit + 2 * j)
                                evict(ps[1][j], e, True, unit + 2 * j + 1)
                    unit += 2 * L2
                else:
                    (eA, posA, LA), (eB, posB, LB), m = payload
                    sides = [(0, eA), (1, eB)]
                    tapsS = {0: list(_tap_offsets(eA)),
                             1: list(_tap_offsets(eB))}
                    stS = {}
                    for rh, e_ in sides:
                        cols = _GEOM[e_]["cols"]
                        rws = _GEOM[e_]["rows"](2)
                        ne = rws * cols
                        src = xin0 if rh == 0 else xin1
                        pool_ = slab_pool if rh == 0 else vc_pool
                        stS[rh] = pool_.tile(
                            [128, rws, cols], dt.bfloat16,
                            name="st" if rh == 0 else "stB")
                        nc.gpsimd.dma_start(
                            out=stS[rh][rh * 64:rh * 64 + 64, :, :],
                            in_=src[:, offs[rh]:offs[rh] + ne])
                    ps = [psum_pool.tile([128, 512], dt.float32,
                                         name=f"psb_{rh}_0")
                          for rh in range(2)]
                    Tmax = max(len(tapsS[0]), len(tapsS[1]))
                    for t in range(Tmax):
                        for rh, e_ in sides:
                            tl = tapsS[rh]
                            if t >= len(tl):
                                continue
                            slot, ro, co = tl[t]
                            first = t == 0
                            last = t == len(tl) - 1
                            for col in range(2):
                                mm_quad(ps[rh], stS[rh], e_, slot, ro, co,
                                        col, rh, col, first, last, False)
                            if last:
                                evict(ps[rh], e_, rh == 1, unit + rh)
                    unit += 2
            assert unit == nunits


def _get_program(key, runseq, tot_elems):
    if key not in _COMPILED:
        _COMPILED[key] = _build_program(runseq, tot_elems)
    return _COMPILED[key]


def _prep_weights(ws, bs, bn_scale, bn_bias, bn_mean, bn_var):
    bf16 = ml_dtypes.bfloat16
    wt = np.empty((CIN, NTAPS, CE), dtype=bf16)
    for e in range(4):
        k = KS[e]
        w = ws[e].astype(np.float32)  # [CE, CIN, k, k]
        wt[:, _SLOT_BASE[e]:_SLOT_BASE[e] + k * k, :] = (
            w.transpose(1, 2, 3, 0).reshape(CIN, k * k, CE).astype(bf16))
    inv = (bn_scale / np.sqrt(bn_var + BN_EPS)).astype(np.float32)
    shift = (np.stack(bs) * inv + bn_bias - bn_mean * inv).astype(np.float32)
    bnp = np.stack([inv, shift], axis=1)              # [4, 2, CE]
    bnp = np.ascontiguousarray(bnp.transpose(2, 0, 1))  # [CE, 4, 2]
    return wt, bnp


def run(inputs, trace=False):
    from concourse import bass_utils

    x = np.asarray(inputs["x"], dtype=np.float32)
    ws = [np.asarray(inputs[f"w{i}"], dtype=np.float32) for i in range(4)]
    bs = [np.asarray(inputs[f"b{i}"], dtype=np.float32) for i in range(4)]
    bn_scale = np.asarray(inputs["bn_scale"], dtype=np.float32)
    bn_bias = np.asarray(inputs["bn_bias"], dtype=np.float32)
    bn_mean = np.asarray(inputs["bn_mean"], dtype=np.float32)
    bn_var = np.asarray(inputs["bn_var"], dtype=np.float32)
    gate_w = np.asarray(inputs["gate_w"], dtype=np.float32)
    gate_b = np.asarray(inputs["gate_b"], dtype=np.float32)

    idx, wsel = _gate(x, gate_w, gate_b)
    key, patterns, runseq, units, assign, tot_elems = _plan(idx)
    nc = _get_program(key, runseq, tot_elems)
    wt, bnp = _prep_weights(ws, bs, bn_scale, bn_bias, bn_mean, bn_var)

    bf16 = ml_dtypes.bfloat16
    xpad = np.zeros((B, CIN, HP, WP), dtype=bf16)
    xpad[:, :, PAD:PAD + H, PAD:PAD + W] = x.astype(bf16)

    def slab(s, c0, e, L):
        g = _GEOM[e]
        rs, base = g["rs"], g["ro_min"]
        rows, cols = g["rows"](L), g["cols"]
        r0 = 8 * c0 + base
        return xpad[s, :, r0:r0 + rows * rs:rs,
                    base:base + cols * rs:rs].reshape(CIN, -1)

    in_maps = []
    for c in range(NCORES):
        xin = [np.zeros((CIN, tot_elems[0]), dtype=bf16),
               np.zeros((CIN, tot_elems[1]), dtype=bf16)]
        for kind, payload, offs in runseq:
            if kind == "n":
                e, pos, L = payload
                ne = _GEOM[e]["rows"](L) * _GEOM[e]["cols"]
                for rh in range(2):
                    s, c0 = assign[(c * 2 + rh, e, pos)]
                    xin[rh][:, offs[rh]:offs[rh] + ne] = slab(s, c0, e, L)
            else:
                (eA, posA, LA), (eB, posB, LB), m = payload
                for rh, (e_, pos_, L_) in ((0, (eA, posA, LA)),
                                           (1, (eB, posB, LB))):
                    ne = _GEOM[e_]["rows"](L_) * _GEOM[e_]["cols"]
                    s, c0 = assign[(c * 2 + m, e_, pos_)]
                    xin[rh][:, offs[rh]:offs[rh] + ne] = slab(s, c0, e_, L_)
        in_maps.append({"xin0": xin[0], "xin1": xin[1],
                        "wt": wt, "bnp": bnp})

    res = bass_utils.run_bass_kernel_spmd(
        nc, in_maps, core_ids=list(range(NCORES)), trace=trace)

    # scatter device outputs -> (sample, rank) feature maps, weight, concat
    rank = {}
    for s in range(B):
        rank[(s, idx[s, 0])] = 0
        rank[(s, idx[s, 1])] = 1
    E = np.zeros((B, 2, CE, HO, WO), dtype=np.float32)
    # per-(e) position counters replicated per core (same unit order)
    for c in range(NCORES):
        o = res.results[c]["out"]  # [128, nunits, 512] bf16
        of = o.astype(np.float32)
        for u, (e, pos, rh, j, L2) in enumerate(units):
            i = c * 2 + rh
            s, c0 = assign[(i, e, pos)]
            r = rank[(s, e)]
            ca, cb = c0 + j, c0 + L2 + j
            E[s, r, :, 4 * ca:4 * ca + 4, :] = of[0:64, u].reshape(64, 4, WO)
            E[s, r, :, 4 * cb:4 * cb + 4, :] = of[64:128, u].reshape(64, 4, WO)
    outf = (E * wsel[:, :, None, None, None]).reshape(B, 2 * CE, HO, WO)
    return np.ascontiguousarray(outf), res


def kernel(**inputs):
    outf, _ = run(inputs, trace=False)
    return outf


# revision 41
# speedup vs baseline: 1.0540x; 1.0162x over previous
"""MoE downsample kernel for 8 TRN2 NeuronCores — top-2 sparse version.

Host computes the gate (cheap 16x64x4 matvec) first, so the device only
computes each sample's two selected experts (~half the dense MACs). The
SPMD constraint (one program on all 8 cores) is satisfied by an
expert-major uniform schedule parameterized only by the per-expert
selection counts n_e: for expert e every row-half instance (8 cores x 2
PE row-halves = 16 instances) runs an identical pattern of chunk-runs
(run lengths = binary decomposition of 2*n_e into {8,4,2}); the host
gathers the right (sample, chunk-range) input slab into each run's slot
and scatters the outputs back.

Per run of L chunks (chunk = 4 output rows = 512 px): the two PE column
halves process L/2 chunks each, tap-outer loop so one LDWEIGHTS per
(tap, quadrant) covers L/2 matmuls (trailing matmuls set ldweights=False
to reuse the loaded stationary operand). BN + conv-bias + GELU fused
into the ScalarE PSUM eviction; outputs written bf16; gate weighting and
top-2 concat on host.
"""

import numpy as np
import ml_dtypes

KS = [3, 5, 7, 9]
DS = [1, 2, 3, 4]
BN_EPS = 1e-5
B, CIN, H, W = 16, 64, 256, 256
CE = 64
PAD = 16
HP = WP = PAD + 256 + 15   # 287
HO = WO = 128
NCORES = 8
NTAPS = sum(k * k for k in KS)  # 164
NCHUNKS = 32                    # 4 output rows per chunk
EXPERT_ORDER = [3, 2, 0, 1]     # heavy first; light-DMA e1 last
USE_LDW_SKIP = True

_SLOT_BASE = np.cumsum([0] + [k * k for k in KS]).tolist()

# Per-expert slab geometry. Experts with even dilation (e1 d=2, e3 d=4)
# only ever read even rows/cols of the padded image (offsets and strides
# all even), so the host pre-decimates those slabs 2x in each dim.
# ro/co ranges: ro = d*u - pad + PAD over u in [0,k).
_GEOM = {}
for _e in range(4):
    _k, _d = KS[_e], DS[_e]
    _pad = _d * (_k - 1) // 2
    _ro = [_d * _u - _pad + PAD for _u in range(_k)]
    _rs = 2 if _d % 2 == 0 else 1      # host decimation factor
    _ro_min, _ro_max = min(_ro), max(_ro)
    # slab rows for a run of L chunks (in decimated units)
    # original rows [ro_min, 8(L-1)+ro_max+6]
    _GEOM[_e] = {
        "rs": _rs,
        "ro_min": _ro_min,
        "rows": lambda L, a=_ro_min, b=_ro_max, r=_rs: (8 * (L - 1) + b + 6 - a) // r + 1,
        "cols": (_ro_max - _ro_min + 2 * (WO - 1)) // _rs + 1,
    }
_MAXROWS = max(_GEOM[e]["rows"](8) for e in range(4))
_MAXCOLS = max(_GEOM[e]["cols"] for e in range(4))

_COMPILED = {}


def _tap_offsets(e):
    """(slot, row_off, col_off) in padded slab coords for expert e."""
    k, d = KS[e], DS[e]
    pad = d * (k - 1) // 2
    for u in range(k):
        for v in range(k):
            slot = _SLOT_BASE[e] + u * k + v
            yield slot, d * u - pad + PAD, d * v - pad + PAD


def _pattern(two_n):
    """Run lengths (each in {2,4,8}, smallest first) summing to 2*n_e."""
    out = []
    if two_n & 2:
        out.append(2)
    if two_n & 4:
        out.append(4)
    out += [8] * ((two_n - sum(out)) // 8)
    assert sum(out) == two_n
    return out


def _gate(x, gate_w, gate_b):
    pooled = x.astype(np.float64).mean(axis=(2, 3)).astype(np.float32)
    logits = pooled @ gate_w.T.astype(np.float32) + gate_b
    z = logits - logits.max(axis=1, keepdims=True)
    ez = np.exp(z.astype(np.float32))
    gates = ez / ez.sum(axis=1, keepdims=True)
    idx = np.argsort(-gates, axis=1, kind="stable")[:, :2]
    wsel = np.take_along_axis(gates, idx, axis=1)
    wsel = wsel / (wsel.sum(axis=1, keepdims=True) + 1e-8)
    return idx, wsel.astype(np.float32)


def _plan(idx):
    """Build the uniform schedule + per-instance run assignment.

    Returns (key, patterns, runseq, units, assign):
      key      — cache key for the compiled program (depends on n only)
      patterns — {e: [run lengths]}
      runseq   — [(e, L, rowoff, rows)] in program order
      units    — [(e, pos, rh, j, L2)] out-unit order (per core)
      assign   — {(instance 0..15, e, pos): (sample, c0)}
    """
    n = [0, 0, 0, 0]
    samples_e = {e: [] for e in range(4)}
    for s in range(B):
        for e in idx[s]:
            n[e] += 1
            samples_e[e].append(s)
    patterns = {e: _pattern(2 * n[e]) for e in range(4)}
    key = tuple(n)

    # global run inventory per (e, L): cut samples into within-sample runs
    assign = {}
    for e in EXPERT_ORDER:
        pat = patterns[e]
        need = {}
        for L in pat:
            need[L] = need.get(L, 0) + 16
        cursor = 0
        inventory = {L: [] for L in need}
        for L in sorted(need, reverse=True):
            ns = need[L] * L // NCHUNKS     # samples consumed at this L
            assert ns * NCHUNKS == need[L] * L
            for _ in range(ns):
                s = samples_e[e][cursor]
                cursor += 1
                for c0 in range(0, NCHUNKS, L):
                    inventory[L].append((s, c0))
        assert cursor == n[e], (e, cursor, n[e])
        ptr = {L: 0 for L in need}
        for pos, L in enumerate(pat):
            for i in range(16):
                assign[(i, e, pos)] = inventory[L][ptr[L]]
                ptr[L] += 1
        for L in need:
            assert ptr[L] == len(inventory[L])

    # Pair leftover small runs (L=2) of two DIFFERENT experts into one
    # "mixed" run: expert A's chunks on PE row-half 0, expert B's on
    # row-half 1 — all four quadrants stay busy and the per-tap LDWEIGHTS
    # serialization overlaps the other expert's matmuls.
    small = [(e, pos, L) for e in EXPERT_ORDER
             for pos, L in enumerate(patterns[e]) if L == 2]
    mixed = []
    used = set()
    while len(small) >= 2 and small[0][0] != small[1][0]:
        a, b = small.pop(0), small.pop(0)
        mixed.append((a, b))
        used.add((a[0], a[1]))
        used.add((b[0], b[1]))

    # runseq entries: (kind, payload, (off0, off1)) — offsets into the
    # flat per-row-half HBM input arrays (elements, per partition).
    #   kind 'n': payload = (e, pos, L)
    #   kind 'm': payload = ((eA, posA, LA), (eB, posB, LB), m)
    runseq = []
    units = []
    offs = [0, 0]
    # mixed (LDWEIGHTS-bound, small-slab) runs first: they double as the
    # HAM warmup and start compute on a tiny first DMA.
    for a, b in mixed:
        # each mixed run consumes one supply-run per side per core; the
        # 16-instance supply needs two mixed runs per core (m = 0, 1).
        # units store m in the rh slot: instance = core*2 + m.
        for m in range(2):
            runseq.append(("m", (a, b, m), tuple(offs)))
            offs[0] += _GEOM[a[0]]["rows"](a[2]) * _GEOM[a[0]]["cols"]
            offs[1] += _GEOM[b[0]]["rows"](b[2]) * _GEOM[b[0]]["cols"]
            units.append((a[0], a[1], m, 0, 1))
            units.append((b[0], b[1], m, 0, 1))
    for e in EXPERT_ORDER:
        for pos, L in enumerate(patterns[e]):
            if (e, pos) in used:
                continue
            elems = _GEOM[e]["rows"](L) * _GEOM[e]["cols"]
            runseq.append(("n", (e, pos, L), tuple(offs)))
            offs[0] += elems
            offs[1] += elems
            L2 = L // 2
            for j in range(L2):
                for rh in range(2):
                    units.append((e, pos, rh, j, L2))
    return key, patterns, runseq, units, assign, tuple(offs)


def _dedup_ldweights(blocks):
    """Drop InstLdweights that reload the stationary operand already in a
    quadrant (same weights AP + tile_position, no intervening load). The
    following matmuls (ldweights=False) then reuse the loaded weights.
    Deps of a dropped load are merged into its paired matmul."""
    ndrop = 0
    for bb, insts in blocks.items():
        last = {}
        keep = []
        i = 0
        while i < len(insts):
            inst = insts[i]
            if type(inst).__name__ == "InstLdweights":
                ap = inst.ins[0]
                tp = str(getattr(inst, "tile_position", None))
                sig = (str(ap), str(getattr(inst, "perf_mode", None)))
                nxt = insts[i + 1] if i + 1 < len(insts) else None
                if (last.get(tp) == sig and nxt is not None
                        and type(nxt).__name__ == "InstMatmult"):
                    nxt.merge_dependencies_from(inst)
                    ndrop += 1
                    i += 1
                    continue
                last[tp] = sig
            keep.append(inst)
            i += 1
        insts[:] = keep
    return ndrop


def _build_program(runseq, tot_elems):
    import concourse.bass as bass  # noqa: F401
    import concourse.mybir as mybir
    import concourse.tile as tile
    from concourse import bacc
    from contextlib import ExitStack

    dt = mybir.dt
    nunits = sum(2 * (r[1][2] // 2) if r[0] == "n" else 2 for r in runseq)

    nc = bacc.Bacc("TRN2", target_bir_lowering=False, debug=False,
                   num_devices=NCORES)
    xin0 = nc.dram_tensor("xin0", [CIN, tot_elems[0]], dt.bfloat16,
                          kind="ExternalInput")
    xin1 = nc.dram_tensor("xin1", [CIN, tot_elems[1]], dt.bfloat16,
                          kind="ExternalInput")
    wt = nc.dram_tensor("wt", [CIN, NTAPS, CE], dt.bfloat16,
                        kind="ExternalInput")
    bnp = nc.dram_tensor("bnp", [CE, 4, 2], dt.float32, kind="ExternalInput")
    out = nc.dram_tensor("out", [128, nunits, 512], dt.bfloat16,
                         kind="ExternalOutput")

    orig_legalize = tile.tile_legalize

    def legalize_and_dedup(blocks, nc_):
        res = orig_legalize(blocks, nc_)
        _dedup_ldweights(res)
        return res

    tile.tile_legalize = legalize_and_dedup
    try:
        _trace_program(nc, tile, mybir, dt, runseq, xin0, xin1, wt, bnp, out)
    finally:
        tile.tile_legalize = orig_legalize

    nc.compile()
    return nc


def _trace_program(nc, tile, mybir, dt, runseq, xin0, xin1, wt, bnp, out):
    from contextlib import ExitStack

    nunits = sum(2 * (r[1][2] // 2) if r[0] == "n" else 2 for r in runseq)
    with tile.TileContext(nc) as tc:
        with ExitStack() as ctx:
            consts = ctx.enter_context(tc.tile_pool(name="consts", bufs=1))
            slab_pool = ctx.enter_context(tc.tile_pool(name="slab", bufs=3))
            vc_pool = ctx.enter_context(tc.tile_pool(name="vc", bufs=1))
            stage_pool = ctx.enter_context(tc.tile_pool(name="st", bufs=8))
            psum_pool = ctx.enter_context(
                tc.tile_pool(name="ps", bufs=1, space="PSUM"))

            wtile = consts.tile([128, NTAPS, CE], dt.bfloat16)
            bntile = consts.tile([128, 4, 2], dt.float32)
            # scalar-queue DMA: weights load in parallel with the first
            # slab on the gpsimd queue, so compute starts sooner.
            for half in range(2):
                p0 = half * 64
                nc.scalar.dma_start(out=wtile[p0:p0 + 64, :, :], in_=wt.ap())
                nc.scalar.dma_start(out=bntile[p0:p0 + 64, :, :],
                                    in_=bnp.ap())

            def evict(ps_tile, e, free_fast, u):
                # free_fast: VectorE copies the bank out (frees it fast,
                # in parallel with ScalarE's gelu chain), ScalarE gelu
                # later from SBUF. Else ScalarE gelu straight from PSUM.
                stg = stage_pool.tile([128, 512], dt.bfloat16, name="stg")
                if not free_fast:
                    nc.scalar.activation(
                        stg, ps_tile,
                        mybir.ActivationFunctionType.Gelu,
                        scale=bntile[:, e, 0:1], bias=bntile[:, e, 1:2])
                else:
                    vc = vc_pool.tile([128, 512], dt.float32,
                                      name=f"vc_{u % 4}")
                    nc.vector.tensor_scalar_add(vc, ps_tile, 0.0)
                    nc.scalar.activation(
                        stg, vc,
                        mybir.ActivationFunctionType.Gelu,
                        scale=bntile[:, e, 0:1], bias=bntile[:, e, 1:2])
                nc.sync.dma_start(out=out[:, u, :], in_=stg)

            def mm_quad(ps_tile, st, e, slot, ro, co, cc, rh, col,
                        first, last, skip_ldw):
                g = _GEOM[e]
                rs, base = g["rs"], g["ro_min"]
                p0, q0 = rh * 64, col * 64
                rl = (8 * cc + ro - base) // rs
                cl = (co - base) // rs
                rstep = 2 // rs
                rhs = st[p0:p0 + 64,
                         rl:rl + 3 * rstep + 1:rstep,
                         cl:cl + (WO - 1) * rstep + 1:rstep]
                mm = nc.tensor.matmul(
                    ps_tile[q0:q0 + 64, :], wtile[p0:p0 + 64, slot, :],
                    rhs, start=first, stop=last, tile_position=(p0, q0))
                if USE_LDW_SKIP and skip_ldw:
                    mm.ldweights = False

            unit = 0
            for kind, payload, offs in runseq:
                if kind == "n":
                    e, pos, L = payload
                    taps = list(_tap_offsets(e))
                    T = len(taps)
                    L2 = L // 2
                    cols = _GEOM[e]["cols"]
                    rows = _GEOM[e]["rows"](L)
                    ne = rows * cols
                    # tight tile: contiguous per-partition DMA block
                    st = slab_pool.tile([128, rows, cols], dt.bfloat16,
                                        name="st")
                    nc.gpsimd.dma_start(
                        out=st[0:64, :, :],
                        in_=xin0[:, offs[0]:offs[0] + ne])
                    nc.gpsimd.dma_start(
                        out=st[64:128, :, :],
                        in_=xin1[:, offs[1]:offs[1] + ne])
                    ps = [[psum_pool.tile([128, 512], dt.float32,
                                          name=f"psb_{rh}_{j}")
                           for j in range(L2)] for rh in range(2)]
                    for t, (slot, ro, co) in enumerate(taps):
                        first = t == 0
                        last = t == T - 1
                        # chunk-index outer, quadrant inner: the PE queue
                        # is in-order; consecutive MMs must hit different
                        # quadrants to keep all four streaming.
                        for j in range(L2):
                            for rh in range(2):
                                for col in range(2):
                                    mm_quad(ps[rh][j], st, e, slot, ro, co,
                                            col * L2 + j, rh, col,
                                            first, last, j > 0)
                            if last:
                                # bank j complete on both row-halves:
                                # evict now so it frees while later banks'
                                # last-tap matmuls still stream. VectorE
                                # frees rh1 + the last rh0 bank (5 banks),
                                # ScalarE the rest — balanced chains.
                                evict(ps[0][j], e, j == L2 - 1 and L2 > 1,
                                      unit + 2 * j)
                                evict(ps[1][j], e, True, unit + 2 * j + 1)
                    unit += 2 * L2
                else:
                    (eA, posA, LA), (eB, posB, LB), m = payload
                    sides = [(0, eA), (1, eB)]
                    tapsS = {0: list(_tap_offsets(eA)),
                             1: list(_tap_offsets(eB))}
                    stS = {}
                    for rh, e_ in sides:
                        cols = _GEOM[e_]["cols"]
                        rws = _GEOM[e_]["rows"](2)
                        ne = rws * cols
                        src = xin0 if rh == 0 else xin1
                        pool_ = slab_pool if rh == 0 else vc_pool
                        stS[rh] = pool_.tile(
                            [128, rws, cols], dt.bfloat16,
                            name="st" if rh == 0 else "stB")
                        nc.gpsimd.dma_start(
                            out=stS[rh][rh * 64:rh * 64 + 64, :, :],
                            in_=src[:, offs[rh]:offs[rh] + ne])
                    ps = [psum_pool.tile([128, 512], dt.float32,
                                         name=f"psb_{rh}_0")
                          for rh in range(2)]
                    Tmax = max(len(tapsS[0]), len(tapsS[1]))
                    for t in range(Tmax):
                        for rh, e_ in sides:
                            tl = tapsS[rh]
                            if t >= len(tl):
                                continue
                            slot, ro, co = tl[t]
                            first = t == 0
                            last = t == len(tl) - 1
                            for col in range(2):
                                mm_quad(ps[rh], stS[rh], e_, slot, ro, co,
                                        col, rh, col, first, last, False)
                            if last:
                                evict(ps[rh], e_, rh == 1, unit + rh)
                    unit += 2
            assert unit == nunits


def _get_program(key, runseq, tot_elems):
    if key not in _COMPILED:
        _COMPILED[key] = _build_program(runseq, tot_elems)
    return _COMPILED[key]


def _prep_weights(ws, bs, bn_scale, bn_bias, bn_mean, bn_var):
    bf16 = ml_dtypes.bfloat16
    wt = np.empty((CIN, NTAPS, CE), dtype=bf16)
    for e in range(4):
        k = KS[e]
        w = ws[e].astype(np.float32)  # [CE, CIN, k, k]
        wt[:, _SLOT_BASE[e]:_SLOT_BASE[e] + k * k, :] = (
            w.transpose(1, 2, 3, 0).reshape(CIN, k * k, CE).astype(bf16))
    inv = (bn_scale / np.sqrt(bn_var + BN_EPS)).astype(np.float32)
    shift = (np.stack(bs) * inv + bn_bias - bn_mean * inv).astype(np.float32)
    bnp = np.stack([inv, shift], axis=1)              # [4, 2, CE]
    bnp = np.ascontiguousarray(bnp.transpose(2, 0, 1))  # [CE, 4, 2]
    return wt, bnp


def run(inputs, trace=False):
    from concourse import bass_utils

    x = np.asarray(inputs["x"], dtype=np.float32)
    ws = [np.asarray(inputs[f"w{i}"], dtype=np.float32) for i in range(4)]
    bs = [np.asarray(inputs[f"b{i}"], dtype=np.float32) for i in range(4)]
    bn_scale = np.asarray(inputs["bn_scale"], dtype=np.float32)
    bn_bias = np.asarray(inputs["bn_bias"], dtype=np.float32)
    bn_mean = np.asarray(inputs["bn_mean"], dtype=np.float32)
    bn_var = np.asarray(inputs["bn_var"], dtype=np.float32)
    gate_w = np.asarray(inputs["gate_w"], dtype=np.float32)
    gate_b = np.asarray(inputs["gate_b"], dtype=np.float32)

    idx, wsel = _gate(x, gate_w, gate_b)
    key, patterns, runseq, units, assign, tot_elems = _plan(idx)
    nc = _get_program(key, runseq, tot_elems)
    wt, bnp = _prep_weights(ws, bs, bn_scale, bn_bias, bn_mean, bn_var)

    bf16 = ml_dtypes.bfloat16
    xpad = np.zeros((B, CIN, HP, WP), dtype=bf16)
    xpad[:, :, PAD:PAD + H, PAD:PAD + W] = x.astype(bf16)

    def slab(s, c0, e, L):
        g = _GEOM[e]
        rs, base = g["rs"], g["ro_min"]
        rows, cols = g["rows"](L), g["cols"]
        r0 = 8 * c0 + base
        return xpad[s, :, r0:r0 + rows * rs:rs,
                    base:base + cols * rs:rs].reshape(CIN, -1)

    in_maps = []
    for c in range(NCORES):
        xin = [np.zeros((CIN, tot_elems[0]), dtype=bf16),
               np.zeros((CIN, tot_elems[1]), dtype=bf16)]
        for kind, payload, offs in runseq:
            if kind == "n":
                e, pos, L = payload
                ne = _GEOM[e]["rows"](L) * _GEOM[e]["cols"]
                for rh in range(2):
                    s, c0 = assign[(c * 2 + rh, e, pos)]
                    xin[rh][:, offs[rh]:offs[rh] + ne] = slab(s, c0, e, L)
            else:
                (eA, posA, LA), (eB, posB, LB), m = payload
                for rh, (e_, pos_, L_) in ((0, (eA, posA, LA)),
                                           (1, (eB, posB, LB))):
                    ne = _GEOM[e_]["rows"](L_) * _GEOM[e_]["cols"]
                    s, c0 = assign[(c * 2 + m, e_, pos_)]
                    xin[rh][:, offs[rh]:offs[rh] + ne] = slab(s, c0, e_, L_)
        in_maps.append({"xin0": xin[0], "xin1": xin[1],
                        "wt": wt, "bnp": bnp})

    res = bass_utils.run_bass_kernel_spmd(
        nc, in_maps, core_ids=list(range(NCORES)), trace=trace)

    # scatter device outputs -> (sample, rank) feature maps, weight, concat
    rank = {}
    for s in range(B):
        rank[(s, idx[s, 0])] = 0
        rank[(s, idx[s, 1])] = 1
    E = np.zeros((B, 2, CE, HO, WO), dtype=np.float32)
    # per-(e) position counters replicated per core (same unit order)
    for c in range(NCORES):
        o = res.results[c]["out"]  # [128, nunits, 512] bf16
        of = o.astype(np.float32)
        for u, (e, pos, rh, j, L2) in enumerate(units):
            i = c * 2 + rh
            s, c0 = assign[(i, e, pos)]
            r = rank[(s, e)]
            ca, cb = c0 + j, c0 + L2 + j
            E[s, r, :, 4 * ca:4 * ca + 4, :] = of[0:64, u].reshape(64, 4, WO)
            E[s, r, :, 4 * cb:4 * cb + 4, :] = of[64:128, u].reshape(64, 4, WO)
    outf = (E * wsel[:, :, None, None, None]).reshape(B, 2 * CE, HO, WO)
    return np.ascontiguousarray(outf), res


def kernel(**inputs):
    outf, _ = run(inputs, trace=False)
    return outf


# revision 47
# speedup vs baseline: 1.0656x; 1.0110x over previous
"""MoE downsample kernel for 8 TRN2 NeuronCores — top-2 sparse version.

Host computes the gate (cheap 16x64x4 matvec) first, so the device only
computes each sample's two selected experts (~half the dense MACs). The
SPMD constraint (one program on all 8 cores) is satisfied by an
expert-major uniform schedule parameterized only by the per-expert
selection counts n_e: for expert e every row-half instance (8 cores x 2
PE row-halves = 16 instances) runs an identical pattern of chunk-runs
(run lengths = binary decomposition of 2*n_e into {8,4,2}); the host
gathers the right (sample, chunk-range) input slab into each run's slot
and scatters the outputs back.

Per run of L chunks (chunk = 4 output rows = 512 px): the two PE column
halves process L/2 chunks each, tap-outer loop so one LDWEIGHTS per
(tap, quadrant) covers L/2 matmuls (trailing matmuls set ldweights=False
to reuse the loaded stationary operand). BN + conv-bias + GELU fused
into the ScalarE PSUM eviction; outputs written bf16; gate weighting and
top-2 concat on host.
"""

import numpy as np
import ml_dtypes

KS = [3, 5, 7, 9]
DS = [1, 2, 3, 4]
BN_EPS = 1e-5
B, CIN, H, W = 16, 64, 256, 256
CE = 64
PAD = 16
HP = WP = PAD + 256 + 15   # 287
HO = WO = 128
NCORES = 8
NTAPS = sum(k * k for k in KS)  # 164
NCHUNKS = 32                    # 4 output rows per chunk
EXPERT_ORDER = [3, 2, 0, 1]     # heavy first; light-DMA e1 last
USE_LDW_SKIP = True

_SLOT_BASE = np.cumsum([0] + [k * k for k in KS]).tolist()

# Per-expert slab geometry. Experts with even dilation (e1 d=2, e3 d=4)
# only ever read even rows/cols of the padded image (offsets and strides
# all even), so the host pre-decimates those slabs 2x in each dim.
# ro/co ranges: ro = d*u - pad + PAD over u in [0,k).
_GEOM = {}
for _e in range(4):
    _k, _d = KS[_e], DS[_e]
    _pad = _d * (_k - 1) // 2
    _ro = [_d * _u - _pad + PAD for _u in range(_k)]
    _rs = 2 if _d % 2 == 0 else 1      # host decimation factor
    _ro_min, _ro_max = min(_ro), max(_ro)
    # slab rows for a run of L chunks (in decimated units)
    # original rows [ro_min, 8(L-1)+ro_max+6]
    _GEOM[_e] = {
        "rs": _rs,
        "ro_min": _ro_min,
        "rows": lambda L, a=_ro_min, b=_ro_max, r=_rs: (8 * (L - 1) + b + 6 - a) // r + 1,
        "cols": (_ro_max - _ro_min + 2 * (WO - 1)) // _rs + 1,
    }
_MAXROWS = max(_GEOM[e]["rows"](8) for e in range(4))
_MAXCOLS = max(_GEOM[e]["cols"] for e in range(4))

_COMPILED = {}


def _tap_offsets(e):
    """(slot, row_off, col_off) in padded slab coords for expert e."""
    k, d = KS[e], DS[e]
    pad = d * (k - 1) // 2
    for u in range(k):
        for v in range(k):
            slot = _SLOT_BASE[e] + u * k + v
            yield slot, d * u - pad + PAD, d * v - pad + PAD


def _pattern(two_n):
    """Run lengths (each in {2,4,8}, smallest first) summing to 2*n_e."""
    out = []
    if two_n & 2:
        out.append(2)
    if two_n & 4:
        out.append(4)
    out += [8] * ((two_n - sum(out)) // 8)
    assert sum(out) == two_n
    return out


def _gate(x, gate_w, gate_b):
    pooled = x.astype(np.float64).mean(axis=(2, 3)).astype(np.float32)
    logits = pooled @ gate_w.T.astype(np.float32) + gate_b
    z = logits - logits.max(axis=1, keepdims=True)
    ez = np.exp(z.astype(np.float32))
    gates = ez / ez.sum(axis=1, keepdims=True)
    idx = np.argsort(-gates, axis=1, kind="stable")[:, :2]
    wsel = np.take_along_axis(gates, idx, axis=1)
    wsel = wsel / (wsel.sum(axis=1, keepdims=True) + 1e-8)
    return idx, wsel.astype(np.float32)


def _plan(idx):
    """Build the uniform schedule + per-instance run assignment.

    Returns (key, patterns, runseq, scatter, assign, tot_elems).
    """
    n = [0, 0, 0, 0]
    samples_e = {e: [] for e in range(4)}
    for s in range(B):
        for e in idx[s]:
            n[e] += 1
            samples_e[e].append(s)
    patterns = {e: _pattern(2 * n[e]) for e in range(4)}
    key = tuple(n)

    # global run inventory per (e, L): cut samples into within-sample runs
    assign = {}
    for e in EXPERT_ORDER:
        pat = patterns[e]
        need = {}
        for L in pat:
            need[L] = need.get(L, 0) + 16
        cursor = 0
        inventory = {L: [] for L in need}
        for L in sorted(need, reverse=True):
            ns = need[L] * L // NCHUNKS     # samples consumed at this L
            assert ns * NCHUNKS == need[L] * L
            for _ in range(ns):
                s = samples_e[e][cursor]
                cursor += 1
                for c0 in range(0, NCHUNKS, L):
                    inventory[L].append((s, c0))
        assert cursor == n[e], (e, cursor, n[e])
        ptr = {L: 0 for L in need}
        for pos, L in enumerate(pat):
            for i in range(16):
                assign[(i, e, pos)] = inventory[L][ptr[L]]
                ptr[L] += 1
        for L in need:
            assert ptr[L] == len(inventory[L])

    # Pair leftover small runs (L=2) of two DIFFERENT experts into one
    # "mixed" run: expert A's chunks on PE row-half 0, expert B's on
    # row-half 1 — all four quadrants stay busy and the per-tap LDWEIGHTS
    # serialization overlaps the other expert's matmuls.
    small = [(e, pos, L) for e in EXPERT_ORDER
             for pos, L in enumerate(patterns[e]) if L == 2]
    mixed = []
    used = set()
    while len(small) >= 2 and small[0][0] != small[1][0]:
        a, b = small.pop(0), small.pop(0)
        mixed.append((a, b))
        used.add((a[0], a[1]))
        used.add((b[0], b[1]))

    # runseq entries: (kind, payload, (off0, off1)) — offsets into the
    # flat per-row-half HBM input arrays (elements, per partition).
    #   kind 'n': payload = (e, pos, L)
    #   kind 'm': payload = ((eA, posA, LA), (eB, posB, LB), m)
    # runseq + out-unit scatter records. scatter: per unit u, two records
    # (u, half, iofs, e, pos, joff): device out[half*64:(half+1)*64, u]
    # holds chunk (assign[(core*2+iofs, e, pos)].c0 + joff) of expert e.
    runseq = []
    scatter = []
    offs = [0, 0]
    unit = 0
    # merged mixed run first (LDWEIGHTS-bound, small slabs): both supply
    # runs per side stacked (pieces 0/1 = instances core*2+0/1), expert A
    # on PE row-half 0, expert B (tap loop staggered) on row-half 1.
    for a, b in mixed:
        runseq.append(("M", (a, b), tuple(offs)))
        offs[0] += 2 * _GEOM[a[0]]["rows"](a[2]) * _GEOM[a[0]]["cols"]
        offs[1] += 2 * _GEOM[b[0]]["rows"](b[2]) * _GEOM[b[0]]["cols"]
        for e_, pos_, L_ in (a, b):
            for j in range(2):
                scatter.append((unit, 0, 0, e_, pos_, j))
                scatter.append((unit, 1, 1, e_, pos_, j))
                unit += 1
    for e in EXPERT_ORDER:
        for pos, L in enumerate(patterns[e]):
            if (e, pos) in used:
                continue
            elems = _GEOM[e]["rows"](L) * _GEOM[e]["cols"]
            runseq.append(("n", (e, pos, L), tuple(offs)))
            offs[0] += elems
            offs[1] += elems
            L2 = L // 2
            for j in range(L2):
                for rh in range(2):
                    scatter.append((unit, 0, rh, e, pos, j))
                    scatter.append((unit, 1, rh, e, pos, L2 + j))
                    unit += 1
    return key, patterns, runseq, scatter, assign, tuple(offs)


def _dedup_ldweights(blocks):
    """Drop InstLdweights that reload the stationary operand already in a
    quadrant (same weights AP + tile_position, no intervening load). The
    following matmuls (ldweights=False) then reuse the loaded weights.
    Deps of a dropped load are merged into its paired matmul."""
    ndrop = 0
    for bb, insts in blocks.items():
        last = {}
        keep = []
        i = 0
        while i < len(insts):
            inst = insts[i]
            if type(inst).__name__ == "InstLdweights":
                ap = inst.ins[0]
                tp = str(getattr(inst, "tile_position", None))
                sig = (str(ap), str(getattr(inst, "perf_mode", None)))
                nxt = insts[i + 1] if i + 1 < len(insts) else None
                if (last.get(tp) == sig and nxt is not None
                        and type(nxt).__name__ == "InstMatmult"):
                    nxt.merge_dependencies_from(inst)
                    ndrop += 1
                    i += 1
                    continue
                last[tp] = sig
            keep.append(inst)
            i += 1
        insts[:] = keep
    return ndrop


def _build_program(runseq, tot_elems):
    import concourse.bass as bass  # noqa: F401
    import concourse.mybir as mybir
    import concourse.tile as tile
    from concourse import bacc
    from contextlib import ExitStack

    dt = mybir.dt
    nunits = sum(2 * (r[1][2] // 2) if r[0] == "n" else 4 for r in runseq)

    nc = bacc.Bacc("TRN2", target_bir_lowering=False, debug=False,
                   num_devices=NCORES)
    xin0 = nc.dram_tensor("xin0", [CIN, tot_elems[0]], dt.bfloat16,
                          kind="ExternalInput")
    xin1 = nc.dram_tensor("xin1", [CIN, tot_elems[1]], dt.bfloat16,
                          kind="ExternalInput")
    wt = nc.dram_tensor("wt", [CIN, NTAPS, CE], dt.bfloat16,
                        kind="ExternalInput")
    bnp = nc.dram_tensor("bnp", [CE, 4, 2], dt.float32, kind="ExternalInput")
    out = nc.dram_tensor("out", [128, nunits, 512], dt.bfloat16,
                         kind="ExternalOutput")

    orig_legalize = tile.tile_legalize

    def legalize_and_dedup(blocks, nc_):
        res = orig_legalize(blocks, nc_)
        _dedup_ldweights(res)
        return res

    tile.tile_legalize = legalize_and_dedup
    try:
        _trace_program(nc, tile, mybir, dt, runseq, xin0, xin1, wt, bnp, out)
    finally:
        tile.tile_legalize = orig_legalize

    nc.compile()
    return nc


def _trace_program(nc, tile, mybir, dt, runseq, xin0, xin1, wt, bnp, out):
    from contextlib import ExitStack

    nunits = sum(2 * (r[1][2] // 2) if r[0] == "n" else 4 for r in runseq)
    with tile.TileContext(nc) as tc:
        with ExitStack() as ctx:
            consts = ctx.enter_context(tc.tile_pool(name="consts", bufs=1))
            slab_pool = ctx.enter_context(tc.tile_pool(name="slab", bufs=3))
            vc_pool = ctx.enter_context(tc.tile_pool(name="vc", bufs=1))
            stage_pool = ctx.enter_context(tc.tile_pool(name="st", bufs=8))
            psum_pool = ctx.enter_context(
                tc.tile_pool(name="ps", bufs=1, space="PSUM"))

            wtile = consts.tile([128, NTAPS, CE], dt.bfloat16)
            bntile = consts.tile([128, 4, 2], dt.float32)
            # scalar-queue DMA: weights load in parallel with the first
            # slab on the gpsimd queue, so compute starts sooner.
            for half in range(2):
                p0 = half * 64
                nc.scalar.dma_start(out=wtile[p0:p0 + 64, :, :], in_=wt.ap())
                nc.scalar.dma_start(out=bntile[p0:p0 + 64, :, :],
                                    in_=bnp.ap())

            def evict(ps_tile, e, free_fast, u):
                # free_fast: VectorE copies the bank out (frees it fast,
                # in parallel with ScalarE's gelu chain), ScalarE gelu
                # later from SBUF. Else ScalarE gelu straight from PSUM.
                stg = stage_pool.tile([128, 512], dt.bfloat16, name="stg")
                if not free_fast:
                    nc.scalar.activation(
                        stg, ps_tile,
                        mybir.ActivationFunctionType.Gelu,
                        scale=bntile[:, e, 0:1], bias=bntile[:, e, 1:2])
                else:
                    vc = vc_pool.tile([128, 512], dt.float32,
                                      name=f"vc_{u % 4}")
                    nc.vector.tensor_scalar_add(vc, ps_tile, 0.0)
                    nc.scalar.activation(
                        stg, vc,
                        mybir.ActivationFunctionType.Gelu,
                        scale=bntile[:, e, 0:1], bias=bntile[:, e, 1:2])
                nc.sync.dma_start(out=out[:, u, :], in_=stg)

            def mm_quad(ps_tile, st, e, slot, ro, co, cc, rh, col,
                        first, last, skip_ldw):
                g = _GEOM[e]
                rs, base = g["rs"], g["ro_min"]
                p0, q0 = rh * 64, col * 64
                rl = (8 * cc + ro - base) // rs
                cl = (co - base) // rs
                rstep = 2 // rs
                rhs = st[p0:p0 + 64,
                         rl:rl + 3 * rstep + 1:rstep,
                         cl:cl + (WO - 1) * rstep + 1:rstep]
                mm = nc.tensor.matmul(
                    ps_tile[q0:q0 + 64, :], wtile[p0:p0 + 64, slot, :],
                    rhs, start=first, stop=last, tile_position=(p0, q0))
                if USE_LDW_SKIP and skip_ldw:
                    mm.ldweights = False

            unit = 0
            for kind, payload, offs in runseq:
                if kind == "n":
                    e, pos, L = payload
                    taps = list(_tap_offsets(e))
                    T = len(taps)
                    L2 = L // 2
                    cols = _GEOM[e]["cols"]
                    rows = _GEOM[e]["rows"](L)
                    ne = rows * cols
                    # tight tile: contiguous per-partition DMA block
                    st = slab_pool.tile([128, rows, cols], dt.bfloat16,
                                        name="st")
                    nc.gpsimd.dma_start(
                        out=st[0:64, :, :],
                        in_=xin0[:, offs[0]:offs[0] + ne])
                    nc.gpsimd.dma_start(
                        out=st[64:128, :, :],
                        in_=xin1[:, offs[1]:offs[1] + ne])
                    ps = [[psum_pool.tile([128, 512], dt.float32,
                                          name=f"psb_{rh}_{j}")
                           for j in range(L2)] for rh in range(2)]
                    for t, (slot, ro, co) in enumerate(taps):
                        first = t == 0
                        last = t == T - 1
                        # chunk-index outer, quadrant inner: the PE queue
                        # is in-order; consecutive MMs must hit different
                        # quadrants to keep all four streaming.
                        for j in range(L2):
                            for rh in range(2):
                                for col in range(2):
                                    mm_quad(ps[rh][j], st, e, slot, ro, co,
                                            col * L2 + j, rh, col,
                                            first, last, j > 0)
                            if last:
                                # bank j complete on both row-halves:
                                # evict now so it frees while later banks'
                                # last-tap matmuls still stream. VectorE
                                # frees rh1 + the last rh0 bank (5 banks),
                                # ScalarE the rest — balanced chains.
                                evict(ps[0][j], e, j == L2 - 1 and L2 > 1,
                                      unit + 2 * j)
                                evict(ps[1][j], e, True, unit + 2 * j + 1)
                    unit += 2 * L2
                else:  # "M": merged mixed run, 2 supply-pieces per side
                    (eA, posA, LA), (eB, posB, LB) = payload
                    sides = [(0, eA), (1, eB)]
                    tapsS = {0: list(_tap_offsets(eA)),
                             1: list(_tap_offsets(eB))}
                    STAG = 16  # delay side B so its slab DMA never blocks
                    stS, rwsS = {}, {}
                    for rh, e_ in sides:
                        cols = _GEOM[e_]["cols"]
                        rws = _GEOM[e_]["rows"](2)
                        rwsS[rh] = rws
                        ne = 2 * rws * cols
                        src = xin0 if rh == 0 else xin1
                        pool_ = slab_pool if rh == 0 else vc_pool
                        stS[rh] = pool_.tile(
                            [128, 2 * rws, cols], dt.bfloat16,
                            name="st" if rh == 0 else "stB")
                        nc.gpsimd.dma_start(
                            out=stS[rh][rh * 64:rh * 64 + 64, :, :],
                            in_=src[:, offs[rh]:offs[rh] + ne])
                    ps = [[psum_pool.tile([128, 512], dt.float32,
                                          name=f"psb_{rh}_{j}")
                           for j in range(2)] for rh in range(2)]
                    Tmax = max(len(tapsS[0]), STAG + len(tapsS[1]))
                    for t in range(Tmax):
                        for rh, e_ in sides:
                            tt = t - STAG * rh
                            tl = tapsS[rh]
                            if tt < 0 or tt >= len(tl):
                                continue
                            slot, ro, co = tl[tt]
                            first = tt == 0
                            last = tt == len(tl) - 1
                            g = _GEOM[e_]
                            rs, base = g["rs"], g["ro_min"]
                            p0 = rh * 64
                            for j in range(2):
                                for col in range(2):
                                    # piece = col (stacked in slab rows)
                                    q0 = col * 64
                                    rl = ((8 * j + ro - base) // rs
                                          + col * rwsS[rh])
                                    cl = (co - base) // rs
                                    rstep = 2 // rs
                                    rhs = stS[rh][
                                        p0:p0 + 64,
                                        rl:rl + 3 * rstep + 1:rstep,
                                        cl:cl + (WO - 1) * rstep + 1:rstep]
                                    mm = nc.tensor.matmul(
                                        ps[rh][j][q0:q0 + 64, :],
                                        wtile[p0:p0 + 64, slot, :], rhs,
                                        start=first, stop=last,
                                        tile_position=(p0, q0))
                                    if USE_LDW_SKIP and j > 0:
                                        mm.ldweights = False
                            if last:
                                evict(ps[rh][0], e_, rh == 1,
                                      unit + 2 * rh)
                                evict(ps[rh][1], e_, rh == 1,
                                      unit + 2 * rh + 1)
                    unit += 4
            assert unit == nunits


def _get_program(key, runseq, tot_elems):
    if key not in _COMPILED:
        _COMPILED[key] = _build_program(runseq, tot_elems)
    return _COMPILED[key]


def _prep_weights(ws, bs, bn_scale, bn_bias, bn_mean, bn_var):
    bf16 = ml_dtypes.bfloat16
    wt = np.empty((CIN, NTAPS, CE), dtype=bf16)
    for e in range(4):
        k = KS[e]
        w = ws[e].astype(np.float32)  # [CE, CIN, k, k]
        wt[:, _SLOT_BASE[e]:_SLOT_BASE[e] + k * k, :] = (
            w.transpose(1, 2, 3, 0).reshape(CIN, k * k, CE).astype(bf16))
    inv = (bn_scale / np.sqrt(bn_var + BN_EPS)).astype(np.float32)
    shift = (np.stack(bs) * inv + bn_bias - bn_mean * inv).astype(np.float32)
    bnp = np.stack([inv, shift], axis=1)              # [4, 2, CE]
    bnp = np.ascontiguousarray(bnp.transpose(2, 0, 1))  # [CE, 4, 2]
    return wt, bnp


def run(inputs, trace=False):
    from concourse import bass_utils

    x = np.asarray(inputs["x"], dtype=np.float32)
    ws = [np.asarray(inputs[f"w{i}"], dtype=np.float32) for i in range(4)]
    bs = [np.asarray(inputs[f"b{i}"], dtype=np.float32) for i in range(4)]
    bn_scale = np.asarray(inputs["bn_scale"], dtype=np.float32)
    bn_bias = np.asarray(inputs["bn_bias"], dtype=np.float32)
    bn_mean = np.asarray(inputs["bn_mean"], dtype=np.float32)
    bn_var = np.asarray(inputs["bn_var"], dtype=np.float32)
    gate_w = np.asarray(inputs["gate_w"], dtype=np.float32)
    gate_b = np.asarray(inputs["gate_b"], dtype=np.float32)

    idx, wsel = _gate(x, gate_w, gate_b)
    key, patterns, runseq, scatter, assign, tot_elems = _plan(idx)
    nc = _get_program(key, runseq, tot_elems)
    wt, bnp = _prep_weights(ws, bs, bn_scale, bn_bias, bn_mean, bn_var)

    bf16 = ml_dtypes.bfloat16
    xpad = np.zeros((B, CIN, HP, WP), dtype=bf16)
    xpad[:, :, PAD:PAD + H, PAD:PAD + W] = x.astype(bf16)

    def slab(s, c0, e, L):
        g = _GEOM[e]
        rs, base = g["rs"], g["ro_min"]
        rows, cols = g["rows"](L), g["cols"]
        r0 = 8 * c0 + base
        return xpad[s, :, r0:r0 + rows * rs:rs,
                    base:base + cols * rs:rs].reshape(CIN, -1)

    in_maps = []
    for c in range(NCORES):
        xin = [np.zeros((CIN, tot_elems[0]), dtype=bf16),
               np.zeros((CIN, tot_elems[1]), dtype=bf16)]
        for kind, payload, offs in runseq:
            if kind == "n":
                e, pos, L = payload
                ne = _GEOM[e]["rows"](L) * _GEOM[e]["cols"]
                for rh in range(2):
                    s, c0 = assign[(c * 2 + rh, e, pos)]
                    xin[rh][:, offs[rh]:offs[rh] + ne] = slab(s, c0, e, L)
            else:  # "M": both supply-pieces stacked per side
                (eA, posA, LA), (eB, posB, LB) = payload
                for rh, (e_, pos_, L_) in ((0, (eA, posA, LA)),
                                           (1, (eB, posB, LB))):
                    ne = _GEOM[e_]["rows"](L_) * _GEOM[e_]["cols"]
                    for m in range(2):
                        s, c0 = assign[(c * 2 + m, e_, pos_)]
                        o0 = offs[rh] + m * ne
                        xin[rh][:, o0:o0 + ne] = slab(s, c0, e_, L_)
        in_maps.append({"xin0": xin[0], "xin1": xin[1],
                        "wt": wt, "bnp": bnp})

    res = bass_utils.run_bass_kernel_spmd(
        nc, in_maps, core_ids=list(range(NCORES)), trace=trace)

    # scatter device outputs -> (sample, rank) feature maps, weight, concat
    rank = {}
    for s in range(B):
        rank[(s, idx[s, 0])] = 0
        rank[(s, idx[s, 1])] = 1
    E = np.zeros((B, 2, CE, HO, WO), dtype=np.float32)
    for c in range(NCORES):
        o = res.results[c]["out"]  # [128, nunits, 512] bf16
        of = o.astype(np.float32)
        for u, half, iofs, e, pos, joff in scatter:
            s, c0 = assign[(c * 2 + iofs, e, pos)]
            r = rank[(s, e)]
            ch = c0 + joff
            E[s, r, :, 4 * ch:4 * ch + 4, :] = (
                of[half * 64:half * 64 + 64, u].reshape(64, 4, WO))
    outf = (E * wsel[:, :, None, None, None]).reshape(B, 2 * CE, HO, WO)
    return np.ascontiguousarray(outf), res


def kernel(**inputs):
    outf, _ = run(inputs, trace=False)
    return outf


# revision 48
# speedup vs baseline: 1.0719x; 1.0059x over previous
"""MoE downsample kernel for 8 TRN2 NeuronCores — top-2 sparse version.

Host computes the gate (cheap 16x64x4 matvec) first, so the device only
computes each sample's two selected experts (~half the dense MACs). The
SPMD constraint (one program on all 8 cores) is satisfied by an
expert-major uniform schedule parameterized only by the per-expert
selection counts n_e: for expert e every row-half instance (8 cores x 2
PE row-halves = 16 instances) runs an identical pattern of chunk-runs
(run lengths = binary decomposition of 2*n_e into {8,4,2}); the host
gathers the right (sample, chunk-range) input slab into each run's slot
and scatters the outputs back.

Per run of L chunks (chunk = 4 output rows = 512 px): the two PE column
halves process L/2 chunks each, tap-outer loop so one LDWEIGHTS per
(tap, quadrant) covers L/2 matmuls (trailing matmuls set ldweights=False
to reuse the loaded stationary operand). BN + conv-bias + GELU fused
into the ScalarE PSUM eviction; outputs written bf16; gate weighting and
top-2 concat on host.
"""

import numpy as np
import ml_dtypes

KS = [3, 5, 7, 9]
DS = [1, 2, 3, 4]
BN_EPS = 1e-5
B, CIN, H, W = 16, 64, 256, 256
CE = 64
PAD = 16
HP = WP = PAD + 256 + 15   # 287
HO = WO = 128
NCORES = 8
NTAPS = sum(k * k for k in KS)  # 164
NCHUNKS = 32                    # 4 output rows per chunk
EXPERT_ORDER = [3, 2, 0, 1]     # heavy first; light-DMA e1 last
USE_LDW_SKIP = True

_SLOT_BASE = np.cumsum([0] + [k * k for k in KS]).tolist()

# Per-expert slab geometry. Experts with even dilation (e1 d=2, e3 d=4)
# only ever read even rows/cols of the padded image (offsets and strides
# all even), so the host pre-decimates those slabs 2x in each dim.
# ro/co ranges: ro = d*u - pad + PAD over u in [0,k).
_GEOM = {}
for _e in range(4):
    _k, _d = KS[_e], DS[_e]
    _pad = _d * (_k - 1) // 2
    _ro = [_d * _u - _pad + PAD for _u in range(_k)]
    _rs = 2 if _d % 2 == 0 else 1      # host decimation factor
    _ro_min, _ro_max = min(_ro), max(_ro)
    # slab rows for a run of L chunks (in decimated units)
    # original rows [ro_min, 8(L-1)+ro_max+6]
    _GEOM[_e] = {
        "rs": _rs,
        "ro_min": _ro_min,
        "rows": lambda L, a=_ro_min, b=_ro_max, r=_rs: (8 * (L - 1) + b + 6 - a) // r + 1,
        "cols": (_ro_max - _ro_min + 2 * (WO - 1)) // _rs + 1,
    }
_MAXROWS = max(_GEOM[e]["rows"](8) for e in range(4))
_MAXCOLS = max(_GEOM[e]["cols"] for e in range(4))

_COMPILED = {}


def _tap_offsets(e):
    """(slot, row_off, col_off) in padded slab coords for expert e."""
    k, d = KS[e], DS[e]
    pad = d * (k - 1) // 2
    for u in range(k):
        for v in range(k):
            slot = _SLOT_BASE[e] + u * k + v
            yield slot, d * u - pad + PAD, d * v - pad + PAD


def _pattern(two_n):
    """Run lengths (each in {2,4,8}, smallest first) summing to 2*n_e."""
    out = []
    if two_n & 2:
        out.append(2)
    if two_n & 4:
        out.append(4)
    out += [8] * ((two_n - sum(out)) // 8)
    assert sum(out) == two_n
    return out


def _gate(x, gate_w, gate_b):
    pooled = x.astype(np.float64).mean(axis=(2, 3)).astype(np.float32)
    logits = pooled @ gate_w.T.astype(np.float32) + gate_b
    z = logits - logits.max(axis=1, keepdims=True)
    ez = np.exp(z.astype(np.float32))
    gates = ez / ez.sum(axis=1, keepdims=True)
    idx = np.argsort(-gates, axis=1, kind="stable")[:, :2]
    wsel = np.take_along_axis(gates, idx, axis=1)
    wsel = wsel / (wsel.sum(axis=1, keepdims=True) + 1e-8)
    return idx, wsel.astype(np.float32)


def _plan(idx):
    """Build the uniform schedule + per-instance run assignment.

    Returns (key, patterns, runseq, scatter, assign, tot_elems).
    """
    n = [0, 0, 0, 0]
    samples_e = {e: [] for e in range(4)}
    for s in range(B):
        for e in idx[s]:
            n[e] += 1
            samples_e[e].append(s)
    patterns = {e: _pattern(2 * n[e]) for e in range(4)}
    key = tuple(n)

    # global run inventory per (e, L): cut samples into within-sample runs
    assign = {}
    for e in EXPERT_ORDER:
        pat = patterns[e]
        need = {}
        for L in pat:
            need[L] = need.get(L, 0) + 16
        cursor = 0
        inventory = {L: [] for L in need}
        for L in sorted(need, reverse=True):
            ns = need[L] * L // NCHUNKS     # samples consumed at this L
            assert ns * NCHUNKS == need[L] * L
            for _ in range(ns):
                s = samples_e[e][cursor]
                cursor += 1
                for c0 in range(0, NCHUNKS, L):
                    inventory[L].append((s, c0))
        assert cursor == n[e], (e, cursor, n[e])
        ptr = {L: 0 for L in need}
        for pos, L in enumerate(pat):
            for i in range(16):
                assign[(i, e, pos)] = inventory[L][ptr[L]]
                ptr[L] += 1
        for L in need:
            assert ptr[L] == len(inventory[L])

    # Pair leftover small runs (L=2) of two DIFFERENT experts into one
    # "mixed" run: expert A's chunks on PE row-half 0, expert B's on
    # row-half 1 — all four quadrants stay busy and the per-tap LDWEIGHTS
    # serialization overlaps the other expert's matmuls.
    small = [(e, pos, L) for e in EXPERT_ORDER
             for pos, L in enumerate(patterns[e]) if L == 2]
    mixed = []
    used = set()
    while len(small) >= 2 and small[0][0] != small[1][0]:
        a, b = small.pop(0), small.pop(0)
        mixed.append((a, b))
        used.add((a[0], a[1]))
        used.add((b[0], b[1]))

    # runseq entries: (kind, payload, (off0, off1)) — offsets into the
    # flat per-row-half HBM input arrays (elements, per partition).
    #   kind 'n': payload = (e, pos, L)
    #   kind 'm': payload = ((eA, posA, LA), (eB, posB, LB), m)
    # runseq + out-unit scatter records. scatter: per unit u, two records
    # (u, half, iofs, e, pos, joff): device out[half*64:(half+1)*64, u]
    # holds chunk (assign[(core*2+iofs, e, pos)].c0 + joff) of expert e.
    runseq = []
    scatter = []
    offs = [0, 0]
    unit = 0
    # merged mixed run first (LDWEIGHTS-bound, small slabs): both supply
    # runs per side stacked (pieces 0/1 = instances core*2+0/1), expert A
    # on PE row-half 0, expert B (tap loop staggered) on row-half 1.
    for a, b in mixed:
        runseq.append(("M", (a, b), tuple(offs)))
        offs[0] += 2 * _GEOM[a[0]]["rows"](a[2]) * _GEOM[a[0]]["cols"]
        offs[1] += 2 * _GEOM[b[0]]["rows"](b[2]) * _GEOM[b[0]]["cols"]
        for e_, pos_, L_ in (a, b):
            for j in range(2):
                scatter.append((unit, 0, 0, e_, pos_, j))
                scatter.append((unit, 1, 1, e_, pos_, j))
                unit += 1
    for e in EXPERT_ORDER:
        for pos, L in enumerate(patterns[e]):
            if (e, pos) in used:
                continue
            elems = _GEOM[e]["rows"](L) * _GEOM[e]["cols"]
            runseq.append(("n", (e, pos, L), tuple(offs)))
            offs[0] += elems
            offs[1] += elems
            L2 = L // 2
            for j in range(L2):
                for rh in range(2):
                    scatter.append((unit, 0, rh, e, pos, j))
                    scatter.append((unit, 1, rh, e, pos, L2 + j))
                    unit += 1
    return key, patterns, runseq, scatter, assign, tuple(offs)


def _dedup_ldweights(blocks):
    """Drop InstLdweights that reload the stationary operand already in a
    quadrant (same weights AP + tile_position, no intervening load). The
    following matmuls (ldweights=False) then reuse the loaded weights.
    Deps of a dropped load are merged into its paired matmul."""
    ndrop = 0
    for bb, insts in blocks.items():
        last = {}
        keep = []
        i = 0
        while i < len(insts):
            inst = insts[i]
            if type(inst).__name__ == "InstLdweights":
                ap = inst.ins[0]
                tp = str(getattr(inst, "tile_position", None))
                sig = (str(ap), str(getattr(inst, "perf_mode", None)))
                nxt = insts[i + 1] if i + 1 < len(insts) else None
                if (last.get(tp) == sig and nxt is not None
                        and type(nxt).__name__ == "InstMatmult"):
                    nxt.merge_dependencies_from(inst)
                    ndrop += 1
                    i += 1
                    continue
                last[tp] = sig
            keep.append(inst)
            i += 1
        insts[:] = keep
    return ndrop


def _build_program(runseq, tot_elems):
    import concourse.bass as bass  # noqa: F401
    import concourse.mybir as mybir
    import concourse.tile as tile
    from concourse import bacc
    from contextlib import ExitStack

    dt = mybir.dt
    nunits = sum(2 * (r[1][2] // 2) if r[0] == "n" else 4 for r in runseq)

    nc = bacc.Bacc("TRN2", target_bir_lowering=False, debug=False,
                   num_devices=NCORES)
    xin0 = nc.dram_tensor("xin0", [CIN, tot_elems[0]], dt.bfloat16,
                          kind="ExternalInput")
    xin1 = nc.dram_tensor("xin1", [CIN, tot_elems[1]], dt.bfloat16,
                          kind="ExternalInput")
    wt = nc.dram_tensor("wt", [CIN, NTAPS, CE], dt.bfloat16,
                        kind="ExternalInput")
    bnp = nc.dram_tensor("bnp", [CE, 4, 2], dt.float32, kind="ExternalInput")
    out = nc.dram_tensor("out", [128, nunits, 512], dt.bfloat16,
                         kind="ExternalOutput")

    orig_legalize = tile.tile_legalize

    def legalize_and_dedup(blocks, nc_):
        res = orig_legalize(blocks, nc_)
        _dedup_ldweights(res)
        return res

    tile.tile_legalize = legalize_and_dedup
    try:
        _trace_program(nc, tile, mybir, dt, runseq, xin0, xin1, wt, bnp, out)
    finally:
        tile.tile_legalize = orig_legalize

    nc.compile()
    return nc


def _trace_program(nc, tile, mybir, dt, runseq, xin0, xin1, wt, bnp, out):
    from contextlib import ExitStack

    nunits = sum(2 * (r[1][2] // 2) if r[0] == "n" else 4 for r in runseq)
    with tile.TileContext(nc) as tc:
        with ExitStack() as ctx:
            consts = ctx.enter_context(tc.tile_pool(name="consts", bufs=1))
            slab_pool = ctx.enter_context(tc.tile_pool(name="slab", bufs=3))
            vc_pool = ctx.enter_context(tc.tile_pool(name="vc", bufs=1))
            stage_pool = ctx.enter_context(tc.tile_pool(name="st", bufs=8))
            psum_pool = ctx.enter_context(
                tc.tile_pool(name="ps", bufs=1, space="PSUM"))

            wtile = consts.tile([128, NTAPS, CE], dt.bfloat16)
            bntile = consts.tile([128, 4, 2], dt.float32)
            # scalar-queue DMA: weights load in parallel with the first
            # slab on the gpsimd queue, so compute starts sooner.
            for half in range(2):
                p0 = half * 64
                nc.scalar.dma_start(out=wtile[p0:p0 + 64, :, :], in_=wt.ap())
                nc.scalar.dma_start(out=bntile[p0:p0 + 64, :, :],
                                    in_=bnp.ap())

            def evict(ps_tile, e, free_fast, u):
                # free_fast: VectorE copies the bank out (frees it fast,
                # in parallel with ScalarE's gelu chain), ScalarE gelu
                # later from SBUF. Else ScalarE gelu straight from PSUM.
                stg = stage_pool.tile([128, 512], dt.bfloat16, name="stg")
                if not free_fast:
                    nc.scalar.activation(
                        stg, ps_tile,
                        mybir.ActivationFunctionType.Gelu,
                        scale=bntile[:, e, 0:1], bias=bntile[:, e, 1:2])
                else:
                    vc = vc_pool.tile([128, 512], dt.float32,
                                      name=f"vc_{u % 4}")
                    nc.vector.tensor_scalar_add(vc, ps_tile, 0.0)
                    nc.scalar.activation(
                        stg, vc,
                        mybir.ActivationFunctionType.Gelu,
                        scale=bntile[:, e, 0:1], bias=bntile[:, e, 1:2])
                nc.sync.dma_start(out=out[:, u, :], in_=stg)

            def mm_quad(ps_tile, st, e, slot, ro, co, cc, rh, col,
                        first, last, skip_ldw):
                g = _GEOM[e]
                rs, base = g["rs"], g["ro_min"]
                p0, q0 = rh * 64, col * 64
                rl = (8 * cc + ro - base) // rs
                cl = (co - base) // rs
                rstep = 2 // rs
                rhs = st[p0:p0 + 64,
                         rl:rl + 3 * rstep + 1:rstep,
                         cl:cl + (WO - 1) * rstep + 1:rstep]
                mm = nc.tensor.matmul(
                    ps_tile[q0:q0 + 64, :], wtile[p0:p0 + 64, slot, :],
                    rhs, start=first, stop=last, tile_position=(p0, q0))
                if USE_LDW_SKIP and skip_ldw:
                    mm.ldweights = False

            unit = 0
            for kind, payload, offs in runseq:
                if kind == "n":
                    e, pos, L = payload
                    taps = list(_tap_offsets(e))
                    T = len(taps)
                    L2 = L // 2
                    cols = _GEOM[e]["cols"]
                    rows = _GEOM[e]["rows"](L)
                    ne = rows * cols
                    # tight tile: contiguous per-partition DMA block
                    st = slab_pool.tile([128, rows, cols], dt.bfloat16,
                                        name="st")
                    nc.gpsimd.dma_start(
                        out=st[0:64, :, :],
                        in_=xin0[:, offs[0]:offs[0] + ne])
                    nc.gpsimd.dma_start(
                        out=st[64:128, :, :],
                        in_=xin1[:, offs[1]:offs[1] + ne])
                    ps = [[psum_pool.tile([128, 512], dt.float32,
                                          name=f"psb_{rh}_{j}")
                           for j in range(L2)] for rh in range(2)]
                    for t, (slot, ro, co) in enumerate(taps):
                        first = t == 0
                        last = t == T - 1
                        # chunk-index outer, quadrant inner: the PE queue
                        # is in-order; consecutive MMs must hit different
                        # quadrants to keep all four streaming.
                        for j in range(L2):
                            for rh in range(2):
                                for col in range(2):
                                    mm_quad(ps[rh][j], st, e, slot, ro, co,
                                            col * L2 + j, rh, col,
                                            first, last, j > 0)
                            if last:
                                # bank j complete on both row-halves:
                                # evict now so it frees while later banks'
                                # last-tap matmuls still stream. VectorE
                                # frees rh1 + the last rh0 bank (5 banks),
                                # ScalarE the rest — balanced chains.
                                evict(ps[0][j], e, j == L2 - 1 and L2 > 1,
                                      unit + 2 * j)
                                evict(ps[1][j], e, True, unit + 2 * j + 1)
                    unit += 2 * L2
                else:  # "M": merged mixed run, 2 supply-pieces per side
                    (eA, posA, LA), (eB, posB, LB) = payload
                    sides = [(0, eA), (1, eB)]
                    tapsS = {0: list(_tap_offsets(eA)),
                             1: list(_tap_offsets(eB))}
                    STAG = 24  # delay side B so its slab DMA never blocks
                    stS, rwsS = {}, {}
                    for rh, e_ in sides:
                        cols = _GEOM[e_]["cols"]
                        rws = _GEOM[e_]["rows"](2)
                        rwsS[rh] = rws
                        ne = 2 * rws * cols
                        src = xin0 if rh == 0 else xin1
                        pool_ = slab_pool if rh == 0 else vc_pool
                        stS[rh] = pool_.tile(
                            [128, 2 * rws, cols], dt.bfloat16,
                            name="st" if rh == 0 else "stB")
                        nc.gpsimd.dma_start(
                            out=stS[rh][rh * 64:rh * 64 + 64, :, :],
                            in_=src[:, offs[rh]:offs[rh] + ne])
                    ps = [[psum_pool.tile([128, 512], dt.float32,
                                          name=f"psb_{rh}_{j}")
                           for j in range(2)] for rh in range(2)]
                    Tmax = max(len(tapsS[0]), STAG + len(tapsS[1]))
                    for t in range(Tmax):
                        for rh, e_ in sides:
                            tt = t - STAG * rh
                            tl = tapsS[rh]
                            if tt < 0 or tt >= len(tl):
                                continue
                            slot, ro, co = tl[tt]
                            first = tt == 0
                            last = tt == len(tl) - 1
                            g = _GEOM[e_]
                            rs, base = g["rs"], g["ro_min"]
                            p0 = rh * 64
                            for j in range(2):
                                for col in range(2):
                                    # piece = col (stacked in slab rows)
                                    q0 = col * 64
                                    rl = ((8 * j + ro - base) // rs
                                          + col * rwsS[rh])
                                    cl = (co - base) // rs
                                    rstep = 2 // rs
                                    rhs = stS[rh][
                                        p0:p0 + 64,
                                        rl:rl + 3 * rstep + 1:rstep,
                                        cl:cl + (WO - 1) * rstep + 1:rstep]
                                    mm = nc.tensor.matmul(
                                        ps[rh][j][q0:q0 + 64, :],
                                        wtile[p0:p0 + 64, slot, :], rhs,
                                        start=first, stop=last,
                                        tile_position=(p0, q0))
                                    if USE_LDW_SKIP and j > 0:
                                        mm.ldweights = False
                            if last:
                                evict(ps[rh][0], e_, rh == 1,
                                      unit + 2 * rh)
                                evict(ps[rh][1], e_, rh == 1,
                                      unit + 2 * rh + 1)
                    unit += 4
            assert unit == nunits


def _get_program(key, runseq, tot_elems):
    if key not in _COMPILED:
        _COMPILED[key] = _build_program(runseq, tot_elems)
    return _COMPILED[key]


def _prep_weights(ws, bs, bn_scale, bn_bias, bn_mean, bn_var):
    bf16 = ml_dtypes.bfloat16
    wt = np.empty((CIN, NTAPS, CE), dtype=bf16)
    for e in range(4):
        k = KS[e]
        w = ws[e].astype(np.float32)  # [CE, CIN, k, k]
        wt[:, _SLOT_BASE[e]:_SLOT_BASE[e] + k * k, :] = (
            w.transpose(1, 2, 3, 0).reshape(CIN, k * k, CE).astype(bf16))
    inv = (bn_scale / np.sqrt(bn_var + BN_EPS)).astype(np.float32)
    shift = (np.stack(bs) * inv + bn_bias - bn_mean * inv).astype(np.float32)
    bnp = np.stack([inv, shift], axis=1)              # [4, 2, CE]
    bnp = np.ascontiguousarray(bnp.transpose(2, 0, 1))  # [CE, 4, 2]
    return wt, bnp


def run(inputs, trace=False):
    from concourse import bass_utils

    x = np.asarray(inputs["x"], dtype=np.float32)
    ws = [np.asarray(inputs[f"w{i}"], dtype=np.float32) for i in range(4)]
    bs = [np.asarray(inputs[f"b{i}"], dtype=np.float32) for i in range(4)]
    bn_scale = np.asarray(inputs["bn_scale"], dtype=np.float32)
    bn_bias = np.asarray(inputs["bn_bias"], dtype=np.float32)
    bn_mean = np.asarray(inputs["bn_mean"], dtype=np.float32)
    bn_var = np.asarray(inputs["bn_var"], dtype=np.float32)
    gate_w = np.asarray(inputs["gate_w"], dtype=np.float32)
    gate_b = np.asarray(inputs["gate_b"], dtype=np.float32)

    idx, wsel = _gate(x, gate_w, gate_b)
    key, patterns, runseq, scatter, assign, tot_elems = _plan(idx)
    nc = _get_program(key, runseq, tot_elems)
    wt, bnp = _prep_weights(ws, bs, bn_scale, bn_bias, bn_mean, bn_var)

    bf16 = ml_dtypes.bfloat16
    xpad = np.zeros((B, CIN, HP, WP), dtype=bf16)
    xpad[:, :, PAD:PAD + H, PAD:PAD + W] = x.astype(bf16)

    def slab(s, c0, e, L):
        g = _GEOM[e]
        rs, base = g["rs"], g["ro_min"]
        rows, cols = g["rows"](L), g["cols"]
        r0 = 8 * c0 + base
        return xpad[s, :, r0:r0 + rows * rs:rs,
                    base:base + cols * rs:rs].reshape(CIN, -1)

    in_maps = []
    for c in range(NCORES):
        xin = [np.zeros((CIN, tot_elems[0]), dtype=bf16),
               np.zeros((CIN, tot_elems[1]), dtype=bf16)]
        for kind, payload, offs in runseq:
            if kind == "n":
                e, pos, L = payload
                ne = _GEOM[e]["rows"](L) * _GEOM[e]["cols"]
                for rh in range(2):
                    s, c0 = assign[(c * 2 + rh, e, pos)]
                    xin[rh][:, offs[rh]:offs[rh] + ne] = slab(s, c0, e, L)
            else:  # "M": both supply-pieces stacked per side
                (eA, posA, LA), (eB, posB, LB) = payload
                for rh, (e_, pos_, L_) in ((0, (eA, posA, LA)),
                                           (1, (eB, posB, LB))):
                    ne = _GEOM[e_]["rows"](L_) * _GEOM[e_]["cols"]
                    for m in range(2):
                        s, c0 = assign[(c * 2 + m, e_, pos_)]
                        o0 = offs[rh] + m * ne
                        xin[rh][:, o0:o0 + ne] = slab(s, c0, e_, L_)
        in_maps.append({"xin0": xin[0], "xin1": xin[1],
                        "wt": wt, "bnp": bnp})

    res = bass_utils.run_bass_kernel_spmd(
        nc, in_maps, core_ids=list(range(NCORES)), trace=trace)

    # scatter device outputs -> (sample, rank) feature maps, weight, concat
    rank = {}
    for s in range(B):
        rank[(s, idx[s, 0])] = 0
        rank[(s, idx[s, 1])] = 1
    E = np.zeros((B, 2, CE, HO, WO), dtype=np.float32)
    for c in range(NCORES):
        o = res.results[c]["out"]  # [128, nunits, 512] bf16
        of = o.astype(np.float32)
        for u, half, iofs, e, pos, joff in scatter:
            s, c0 = assign[(c * 2 + iofs, e, pos)]
            r = rank[(s, e)]
            ch = c0 + joff
            E[s, r, :, 4 * ch:4 * ch + 4, :] = (
                of[half * 64:half * 64 + 64, u].reshape(64, 4, WO))
    outf = (E * wsel[:, :, None, None, None]).reshape(B, 2 * CE, HO, WO)
    return np.ascontiguousarray(outf), res


def kernel(**inputs):
    outf, _ = run(inputs, trace=False)
    return outf


# revision 49
# speedup vs baseline: 1.0950x; 1.0215x over previous
"""MoE downsample kernel for 8 TRN2 NeuronCores — top-2 sparse version.

Host computes the gate (cheap 16x64x4 matvec) first, so the device only
computes each sample's two selected experts (~half the dense MACs). The
SPMD constraint (one program on all 8 cores) is satisfied by an
expert-major uniform schedule parameterized only by the per-expert
selection counts n_e: for expert e every row-half instance (8 cores x 2
PE row-halves = 16 instances) runs an identical pattern of chunk-runs
(run lengths = binary decomposition of 2*n_e into {8,4,2}); the host
gathers the right (sample, chunk-range) input slab into each run's slot
and scatters the outputs back.

Per run of L chunks (chunk = 4 output rows = 512 px): the two PE column
halves process L/2 chunks each, tap-outer loop so one LDWEIGHTS per
(tap, quadrant) covers L/2 matmuls (trailing matmuls set ldweights=False
to reuse the loaded stationary operand). BN + conv-bias + GELU fused
into the ScalarE PSUM eviction; outputs written bf16; gate weighting and
top-2 concat on host.
"""

import numpy as np
import ml_dtypes

KS = [3, 5, 7, 9]
DS = [1, 2, 3, 4]
BN_EPS = 1e-5
B, CIN, H, W = 16, 64, 256, 256
CE = 64
PAD = 16
HP = WP = PAD + 256 + 15   # 287
HO = WO = 128
NCORES = 8
NTAPS = sum(k * k for k in KS)  # 164
NCHUNKS = 32                    # 4 output rows per chunk
EXPERT_ORDER = [3, 2, 0, 1]     # heavy first; light-DMA e1 last
USE_LDW_SKIP = True

_SLOT_BASE = np.cumsum([0] + [k * k for k in KS]).tolist()

# Per-expert slab geometry. Experts with even dilation (e1 d=2, e3 d=4)
# only ever read even rows/cols of the padded image (offsets and strides
# all even), so the host pre-decimates those slabs 2x in each dim.
# ro/co ranges: ro = d*u - pad + PAD over u in [0,k).
_GEOM = {}
for _e in range(4):
    _k, _d = KS[_e], DS[_e]
    _pad = _d * (_k - 1) // 2
    _ro = [_d * _u - _pad + PAD for _u in range(_k)]
    _rs = 2 if _d % 2 == 0 else 1      # host decimation factor
    _ro_min, _ro_max = min(_ro), max(_ro)
    # slab rows for a run of L chunks (in decimated units)
    # original rows [ro_min, 8(L-1)+ro_max+6]
    _GEOM[_e] = {
        "rs": _rs,
        "ro_min": _ro_min,
        "rows": lambda L, a=_ro_min, b=_ro_max, r=_rs: (8 * (L - 1) + b + 6 - a) // r + 1,
        "cols": (_ro_max - _ro_min + 2 * (WO - 1)) // _rs + 1,
    }
_MAXROWS = max(_GEOM[e]["rows"](8) for e in range(4))
_MAXCOLS = max(_GEOM[e]["cols"] for e in range(4))

_COMPILED = {}


def _tap_offsets(e):
    """(slot, row_off, col_off) in padded slab coords for expert e."""
    k, d = KS[e], DS[e]
    pad = d * (k - 1) // 2
    for u in range(k):
        for v in range(k):
            slot = _SLOT_BASE[e] + u * k + v
            yield slot, d * u - pad + PAD, d * v - pad + PAD


def _pattern(two_n):
    """Run lengths (each in {2,4,8}, smallest first) summing to 2*n_e."""
    out = []
    if two_n & 2:
        out.append(2)
    if two_n & 4:
        out.append(4)
    out += [8] * ((two_n - sum(out)) // 8)
    assert sum(out) == two_n
    return out


def _gate(x, gate_w, gate_b):
    pooled = x.astype(np.float64).mean(axis=(2, 3)).astype(np.float32)
    logits = pooled @ gate_w.T.astype(np.float32) + gate_b
    z = logits - logits.max(axis=1, keepdims=True)
    ez = np.exp(z.astype(np.float32))
    gates = ez / ez.sum(axis=1, keepdims=True)
    idx = np.argsort(-gates, axis=1, kind="stable")[:, :2]
    wsel = np.take_along_axis(gates, idx, axis=1)
    wsel = wsel / (wsel.sum(axis=1, keepdims=True) + 1e-8)
    return idx, wsel.astype(np.float32)


def _plan(idx):
    """Build the uniform schedule + per-instance run assignment.

    Returns (key, patterns, runseq, scatter, assign, tot_elems).
    """
    n = [0, 0, 0, 0]
    samples_e = {e: [] for e in range(4)}
    for s in range(B):
        for e in idx[s]:
            n[e] += 1
            samples_e[e].append(s)
    patterns = {e: _pattern(2 * n[e]) for e in range(4)}
    key = tuple(n)

    # global run inventory per (e, L): cut samples into within-sample runs
    assign = {}
    for e in EXPERT_ORDER:
        pat = patterns[e]
        need = {}
        for L in pat:
            need[L] = need.get(L, 0) + 16
        cursor = 0
        inventory = {L: [] for L in need}
        for L in sorted(need, reverse=True):
            ns = need[L] * L // NCHUNKS     # samples consumed at this L
            assert ns * NCHUNKS == need[L] * L
            for _ in range(ns):
                s = samples_e[e][cursor]
                cursor += 1
                for c0 in range(0, NCHUNKS, L):
                    inventory[L].append((s, c0))
        assert cursor == n[e], (e, cursor, n[e])
        ptr = {L: 0 for L in need}
        for pos, L in enumerate(pat):
            for i in range(16):
                assign[(i, e, pos)] = inventory[L][ptr[L]]
                ptr[L] += 1
        for L in need:
            assert ptr[L] == len(inventory[L])

    # Pair leftover small runs (L=2) of two DIFFERENT experts into one
    # "mixed" run: expert A's chunks on PE row-half 0, expert B's on
    # row-half 1 — all four quadrants stay busy and the per-tap LDWEIGHTS
    # serialization overlaps the other expert's matmuls.
    small = [(e, pos, L) for e in EXPERT_ORDER
             for pos, L in enumerate(patterns[e]) if L == 2]
    mixed = []
    used = set()
    while len(small) >= 2 and small[0][0] != small[1][0]:
        a, b = small.pop(0), small.pop(0)
        mixed.append((a, b))
        used.add((a[0], a[1]))
        used.add((b[0], b[1]))

    # runseq entries: (kind, payload, (off0, off1)) — offsets into the
    # flat per-row-half HBM input arrays (elements, per partition).
    #   kind 'n': payload = (e, pos, L)
    #   kind 'm': payload = ((eA, posA, LA), (eB, posB, LB), m)
    # runseq + out-unit scatter records. scatter: per unit u, two records
    # (u, half, iofs, e, pos, joff): device out[half*64:(half+1)*64, u]
    # holds chunk (assign[(core*2+iofs, e, pos)].c0 + joff) of expert e.
    runseq = []
    scatter = []
    offs = [0, 0]
    unit = 0
    # merged mixed run first (LDWEIGHTS-bound, small slabs): both supply
    # runs per side stacked (pieces 0/1 = instances core*2+0/1), expert A
    # on PE row-half 0, expert B (tap loop staggered) on row-half 1.
    for a, b in mixed:
        runseq.append(("M", (a, b), tuple(offs)))
        offs[0] += 2 * _GEOM[a[0]]["rows"](a[2]) * _GEOM[a[0]]["cols"]
        offs[1] += 2 * _GEOM[b[0]]["rows"](b[2]) * _GEOM[b[0]]["cols"]
        for e_, pos_, L_ in (a, b):
            for j in range(2):
                scatter.append((unit, 0, 0, e_, pos_, j))
                scatter.append((unit, 1, 1, e_, pos_, j))
                unit += 1
    for e in EXPERT_ORDER:
        for pos, L in enumerate(patterns[e]):
            if (e, pos) in used:
                continue
            elems = _GEOM[e]["rows"](L) * _GEOM[e]["cols"]
            runseq.append(("n", (e, pos, L), tuple(offs)))
            offs[0] += elems
            offs[1] += elems
            L2 = L // 2
            for j in range(L2):
                for rh in range(2):
                    scatter.append((unit, 0, rh, e, pos, j))
                    scatter.append((unit, 1, rh, e, pos, L2 + j))
                    unit += 1
    return key, patterns, runseq, scatter, assign, tuple(offs)


def _dedup_ldweights(blocks):
    """Drop InstLdweights that reload the stationary operand already in a
    quadrant (same weights AP + tile_position, no intervening load). The
    following matmuls (ldweights=False) then reuse the loaded weights.
    Deps of a dropped load are merged into its paired matmul."""
    ndrop = 0
    for bb, insts in blocks.items():
        last = {}
        keep = []
        i = 0
        while i < len(insts):
            inst = insts[i]
            if type(inst).__name__ == "InstLdweights":
                ap = inst.ins[0]
                tp = str(getattr(inst, "tile_position", None))
                sig = (str(ap), str(getattr(inst, "perf_mode", None)))
                nxt = insts[i + 1] if i + 1 < len(insts) else None
                if (last.get(tp) == sig and nxt is not None
                        and type(nxt).__name__ == "InstMatmult"):
                    nxt.merge_dependencies_from(inst)
                    ndrop += 1
                    i += 1
                    continue
                last[tp] = sig
            keep.append(inst)
            i += 1
        insts[:] = keep
    return ndrop


def _build_program(runseq, tot_elems):
    import concourse.bass as bass  # noqa: F401
    import concourse.mybir as mybir
    import concourse.tile as tile
    from concourse import bacc
    from contextlib import ExitStack

    dt = mybir.dt
    nunits = sum(2 * (r[1][2] // 2) if r[0] == "n" else 4 for r in runseq)

    nc = bacc.Bacc("TRN2", target_bir_lowering=False, debug=False,
                   num_devices=NCORES)
    xin0 = nc.dram_tensor("xin0", [CIN, tot_elems[0]], dt.bfloat16,
                          kind="ExternalInput")
    xin1 = nc.dram_tensor("xin1", [CIN, tot_elems[1]], dt.bfloat16,
                          kind="ExternalInput")
    wt = nc.dram_tensor("wt", [CIN, NTAPS, CE], dt.bfloat16,
                        kind="ExternalInput")
    bnp = nc.dram_tensor("bnp", [CE, 4, 2], dt.float32, kind="ExternalInput")
    out = nc.dram_tensor("out", [128, nunits, 512], dt.bfloat16,
                         kind="ExternalOutput")

    orig_legalize = tile.tile_legalize

    def legalize_and_dedup(blocks, nc_):
        res = orig_legalize(blocks, nc_)
        _dedup_ldweights(res)
        return res

    tile.tile_legalize = legalize_and_dedup
    try:
        _trace_program(nc, tile, mybir, dt, runseq, xin0, xin1, wt, bnp, out)
    finally:
        tile.tile_legalize = orig_legalize

    nc.compile()
    return nc


def _trace_program(nc, tile, mybir, dt, runseq, xin0, xin1, wt, bnp, out):
    from contextlib import ExitStack

    nunits = sum(2 * (r[1][2] // 2) if r[0] == "n" else 4 for r in runseq)
    with tile.TileContext(nc) as tc:
        with ExitStack() as ctx:
            consts = ctx.enter_context(tc.tile_pool(name="consts", bufs=1))
            slab_pool = ctx.enter_context(tc.tile_pool(name="slab", bufs=3))
            vc_pool = ctx.enter_context(tc.tile_pool(name="vc", bufs=1))
            stage_pool = ctx.enter_context(tc.tile_pool(name="st", bufs=8))
            psum_pool = ctx.enter_context(
                tc.tile_pool(name="ps", bufs=1, space="PSUM"))

            wtile = consts.tile([128, NTAPS, CE], dt.bfloat16)
            bntile = consts.tile([128, 4, 2], dt.float32)
            # scalar-queue DMA: weights load in parallel with the first
            # slab on the gpsimd queue, so compute starts sooner.
            for half in range(2):
                p0 = half * 64
                nc.scalar.dma_start(out=wtile[p0:p0 + 64, :, :], in_=wt.ap())
                nc.scalar.dma_start(out=bntile[p0:p0 + 64, :, :],
                                    in_=bnp.ap())

            def evict(ps_tile, e, free_fast, u):
                # free_fast: VectorE copies the bank out (frees it fast,
                # in parallel with ScalarE's gelu chain), ScalarE gelu
                # later from SBUF. Else ScalarE gelu straight from PSUM.
                stg = stage_pool.tile([128, 512], dt.bfloat16, name="stg")
                if not free_fast:
                    nc.scalar.activation(
                        stg, ps_tile,
                        mybir.ActivationFunctionType.Gelu,
                        scale=bntile[:, e, 0:1], bias=bntile[:, e, 1:2])
                else:
                    vc = vc_pool.tile([128, 512], dt.float32,
                                      name=f"vc_{u % 4}")
                    nc.vector.tensor_scalar_add(vc, ps_tile, 0.0)
                    nc.scalar.activation(
                        stg, vc,
                        mybir.ActivationFunctionType.Gelu,
                        scale=bntile[:, e, 0:1], bias=bntile[:, e, 1:2])
                nc.sync.dma_start(out=out[:, u, :], in_=stg)

            def mm_quad(ps_tile, st, e, slot, ro, co, cc, rh, col,
                        first, last, skip_ldw):
                g = _GEOM[e]
                rs, base = g["rs"], g["ro_min"]
                p0, q0 = rh * 64, col * 64
                rl = (8 * cc + ro - base) // rs
                cl = (co - base) // rs
                rstep = 2 // rs
                rhs = st[p0:p0 + 64,
                         rl:rl + 3 * rstep + 1:rstep,
                         cl:cl + (WO - 1) * rstep + 1:rstep]
                mm = nc.tensor.matmul(
                    ps_tile[q0:q0 + 64, :], wtile[p0:p0 + 64, slot, :],
                    rhs, start=first, stop=last, tile_position=(p0, q0))
                if USE_LDW_SKIP and skip_ldw:
                    mm.ldweights = False

            unit = 0
            for kind, payload, offs in runseq:
                if kind == "n":
                    e, pos, L = payload
                    taps = list(_tap_offsets(e))
                    T = len(taps)
                    L2 = L // 2
                    cols = _GEOM[e]["cols"]
                    rows = _GEOM[e]["rows"](L)
                    ne = rows * cols
                    # tight tile: contiguous per-partition DMA block
                    st = slab_pool.tile([128, rows, cols], dt.bfloat16,
                                        name="st")
                    nc.gpsimd.dma_start(
                        out=st[0:64, :, :],
                        in_=xin0[:, offs[0]:offs[0] + ne])
                    nc.gpsimd.dma_start(
                        out=st[64:128, :, :],
                        in_=xin1[:, offs[1]:offs[1] + ne])
                    ps = [[psum_pool.tile([128, 512], dt.float32,
                                          name=f"psb_{rh}_{j}")
                           for j in range(L2)] for rh in range(2)]
                    # rh1 runs TOFF taps behind rh0: at run boundaries the
                    # next run's rh0 matmuls overlap this run's rh1 tail
                    # (different quadrants), and each row-half's 4-bank
                    # evict chain gets a wider PSUM-reuse window.
                    TOFF = 2
                    for t in range(T + TOFF):
                        for rh in range(2):
                            tt = t - TOFF * rh
                            if tt < 0 or tt >= T:
                                continue
                            slot, ro, co = taps[tt]
                            first = tt == 0
                            last = tt == T - 1
                            for j in range(L2):
                                for col in range(2):
                                    mm_quad(ps[rh][j], st, e, slot, ro, co,
                                            col * L2 + j, rh, col,
                                            first, last, j > 0)
                                if last:
                                    # bank complete: evict immediately so
                                    # it frees while later banks' last-tap
                                    # matmuls still stream. rh0 -> ScalarE
                                    # gelu from PSUM, rh1 -> VectorE copy
                                    # (parallel chains).
                                    evict(ps[rh][j], e, rh == 1,
                                          unit + 2 * j + rh)
                    unit += 2 * L2
                else:  # "M": merged mixed run, 2 supply-pieces per side
                    (eA, posA, LA), (eB, posB, LB) = payload
                    sides = [(0, eA), (1, eB)]
                    tapsS = {0: list(_tap_offsets(eA)),
                             1: list(_tap_offsets(eB))}
                    STAG = 24  # delay side B so its slab DMA never blocks
                    stS, rwsS = {}, {}
                    for rh, e_ in sides:
                        cols = _GEOM[e_]["cols"]
                        rws = _GEOM[e_]["rows"](2)
                        rwsS[rh] = rws
                        ne = 2 * rws * cols
                        src = xin0 if rh == 0 else xin1
                        pool_ = slab_pool if rh == 0 else vc_pool
                        stS[rh] = pool_.tile(
                            [128, 2 * rws, cols], dt.bfloat16,
                            name="st" if rh == 0 else "stB")
                        nc.gpsimd.dma_start(
                            out=stS[rh][rh * 64:rh * 64 + 64, :, :],
                            in_=src[:, offs[rh]:offs[rh] + ne])
                    ps = [[psum_pool.tile([128, 512], dt.float32,
                                          name=f"psb_{rh}_{j}")
                           for j in range(2)] for rh in range(2)]
                    Tmax = max(len(tapsS[0]), STAG + len(tapsS[1]))
                    for t in range(Tmax):
                        for rh, e_ in sides:
                            tt = t - STAG * rh
                            tl = tapsS[rh]
                            if tt < 0 or tt >= len(tl):
                                continue
                            slot, ro, co = tl[tt]
                            first = tt == 0
                            last = tt == len(tl) - 1
                            g = _GEOM[e_]
                            rs, base = g["rs"], g["ro_min"]
                            p0 = rh * 64
                            for j in range(2):
                                for col in range(2):
                                    # piece = col (stacked in slab rows)
                                    q0 = col * 64
                                    rl = ((8 * j + ro - base) // rs
                                          + col * rwsS[rh])
                                    cl = (co - base) // rs
                                    rstep = 2 // rs
                                    rhs = stS[rh][
                                        p0:p0 + 64,
                                        rl:rl + 3 * rstep + 1:rstep,
                                        cl:cl + (WO - 1) * rstep + 1:rstep]
                                    mm = nc.tensor.matmul(
                                        ps[rh][j][q0:q0 + 64, :],
                                        wtile[p0:p0 + 64, slot, :], rhs,
                                        start=first, stop=last,
                                        tile_position=(p0, q0))
                                    if USE_LDW_SKIP and j > 0:
                                        mm.ldweights = False
                            if last:
                                evict(ps[rh][0], e_, rh == 1,
                                      unit + 2 * rh)
                                evict(ps[rh][1], e_, rh == 1,
                                      unit + 2 * rh + 1)
                    unit += 4
            assert unit == nunits


def _get_program(key, runseq, tot_elems):
    if key not in _COMPILED:
        _COMPILED[key] = _build_program(runseq, tot_elems)
    return _COMPILED[key]


def _prep_weights(ws, bs, bn_scale, bn_bias, bn_mean, bn_var):
    bf16 = ml_dtypes.bfloat16
    wt = np.empty((CIN, NTAPS, CE), dtype=bf16)
    for e in range(4):
        k = KS[e]
        w = ws[e].astype(np.float32)  # [CE, CIN, k, k]
        wt[:, _SLOT_BASE[e]:_SLOT_BASE[e] + k * k, :] = (
            w.transpose(1, 2, 3, 0).reshape(CIN, k * k, CE).astype(bf16))
    inv = (bn_scale / np.sqrt(bn_var + BN_EPS)).astype(np.float32)
    shift = (np.stack(bs) * inv + bn_bias - bn_mean * inv).astype(np.float32)
    bnp = np.stack([inv, shift], axis=1)              # [4, 2, CE]
    bnp = np.ascontiguousarray(bnp.transpose(2, 0, 1))  # [CE, 4, 2]
    return wt, bnp


def run(inputs, trace=False):
    from concourse import bass_utils

    x = np.asarray(inputs["x"], dtype=np.float32)
    ws = [np.asarray(inputs[f"w{i}"], dtype=np.float32) for i in range(4)]
    bs = [np.asarray(inputs[f"b{i}"], dtype=np.float32) for i in range(4)]
    bn_scale = np.asarray(inputs["bn_scale"], dtype=np.float32)
    bn_bias = np.asarray(inputs["bn_bias"], dtype=np.float32)
    bn_mean = np.asarray(inputs["bn_mean"], dtype=np.float32)
    bn_var = np.asarray(inputs["bn_var"], dtype=np.float32)
    gate_w = np.asarray(inputs["gate_w"], dtype=np.float32)
    gate_b = np.asarray(inputs["gate_b"], dtype=np.float32)

    idx, wsel = _gate(x, gate_w, gate_b)
    key, patterns, runseq, scatter, assign, tot_elems = _plan(idx)
    nc = _get_program(key, runseq, tot_elems)
    wt, bnp = _prep_weights(ws, bs, bn_scale, bn_bias, bn_mean, bn_var)

    bf16 = ml_dtypes.bfloat16
    xpad = np.zeros((B, CIN, HP, WP), dtype=bf16)
    xpad[:, :, PAD:PAD + H, PAD:PAD + W] = x.astype(bf16)

    def slab(s, c0, e, L):
        g = _GEOM[e]
        rs, base = g["rs"], g["ro_min"]
        rows, cols = g["rows"](L), g["cols"]
        r0 = 8 * c0 + base
        return xpad[s, :, r0:r0 + rows * rs:rs,
                    base:base + cols * rs:rs].reshape(CIN, -1)

    in_maps = []
    for c in range(NCORES):
        xin = [np.zeros((CIN, tot_elems[0]), dtype=bf16),
               np.zeros((CIN, tot_elems[1]), dtype=bf16)]
        for kind, payload, offs in runseq:
            if kind == "n":
                e, pos, L = payload
                ne = _GEOM[e]["rows"](L) * _GEOM[e]["cols"]
                for rh in range(2):
                    s, c0 = assign[(c * 2 + rh, e, pos)]
                    xin[rh][:, offs[rh]:offs[rh] + ne] = slab(s, c0, e, L)
            else:  # "M": both supply-pieces stacked per side
                (eA, posA, LA), (eB, posB, LB) = payload
                for rh, (e_, pos_, L_) in ((0, (eA, posA, LA)),
                                           (1, (eB, posB, LB))):
                    ne = _GEOM[e_]["rows"](L_) * _GEOM[e_]["cols"]
                    for m in range(2):
                        s, c0 = assign[(c * 2 + m, e_, pos_)]
                        o0 = offs[rh] + m * ne
                        xin[rh][:, o0:o0 + ne] = slab(s, c0, e_, L_)
        in_maps.append({"xin0": xin[0], "xin1": xin[1],
                        "wt": wt, "bnp": bnp})

    res = bass_utils.run_bass_kernel_spmd(
        nc, in_maps, core_ids=list(range(NCORES)), trace=trace)

    # scatter device outputs -> (sample, rank) feature maps, weight, concat
    rank = {}
    for s in range(B):
        rank[(s, idx[s, 0])] = 0
        rank[(s, idx[s, 1])] = 1
    E = np.zeros((B, 2, CE, HO, WO), dtype=np.float32)
    for c in range(NCORES):
        o = res.results[c]["out"]  # [128, nunits, 512] bf16
        of = o.astype(np.float32)
        for u, half, iofs, e, pos, joff in scatter:
            s, c0 = assign[(c * 2 + iofs, e, pos)]
            r = rank[(s, e)]
            ch = c0 + joff
            E[s, r, :, 4 * ch:4 * ch + 4, :] = (
                of[half * 64:half * 64 + 64, u].reshape(64, 4, WO))
    outf = (E * wsel[:, :, None, None, None]).reshape(B, 2 * CE, HO, WO)
    return np.ascontiguousarray(outf), res


def kernel(**inputs):
    outf, _ = run(inputs, trace=False)
    return outf
